# revision 17
# baseline (speedup 1.0000x reference)
"""Trainium2 Bass kernel for nn_DecoderBlock (dense transformer block).

Sharding: 8 NeuronCores = 4 batch elements x 2 sequence halves.
Each core computes layer-1 (causal MHA + LN + residual) for its whole batch
element (duplicated across the pair - avoids any collective), then layer-2
full attention + FFN only for its own 512-query half, selected with a
partition-id-driven dynamic slice.  Activations are kept feature-major
[d, s] so every projection is a plain (weights-stationary) matmul; softmax
runs on transposed scores [k, q] with denominators obtained for free from an
extra ones-column in V; LayerNorm stats (over the partition axis) come from
ones-vector matmuls on the PE.

All matmul operands are f16 (fp32 accumulation in PSUM); LN / softmax scalar
math stays fp32.  Expected end-to-end relative error vs the fp32 reference:
~1e-3.
"""
import sys
sys.path.insert(0, '/opt/trn_rl_repo')
import numpy as np
from contextlib import ExitStack

import concourse.bass as bass
import concourse.mybir as mybir
from concourse.tile import TileContext
from concourse.bass2jax import _bass_exec_p, partition_id_tensor, install_neuronx_cc_hook

f32 = mybir.dt.float32
f16 = mybir.dt.float16
AF = mybir.ActivationFunctionType

D = 1024          # d_model
S = 1024          # sequence length
B = 4             # batch
H = 16            # heads
DH = 64           # head dim
F = 4096          # ffn hidden
P = 128
KT = D // P       # 8 d-tiles
SH = 512          # per-core sequence half
EPS = 1e-5
N_CORES = 8


# ---------------------------------------------------------------------------
# walrus workaround: split multi-sem-wait instructions into single-wait NOPs
# ---------------------------------------------------------------------------
def _split_multi_waits(nc, max_waits=1):
    n_split = 0
    for fn in nc.m.functions:
        for bb in fn.blocks:
            new_insts = []
            for inst in bb.instructions:
                si = inst.sync_info
                waits = list(si.on_wait) if si is not None else []
                if len(waits) > max_waits:
                    keep = waits[-max_waits:]
                    for w in waits[:-max_waits]:
                        nop = mybir.InstNoOp(
                            name=nc.get_next_instruction_name(),
                            engine=inst.engine,
                            sync_info=mybir.SyncInfo(on_wait=[w], on_update=[]),
                            bass_nofuse=True,
                        )
                        nc.register_instruction(nop)
                        new_insts.append(nop)
                    inst.sync_info = mybir.SyncInfo(
                        on_wait=keep, on_update=list(si.on_update))
                    n_split += 1
                new_insts.append(inst)
            bb.instructions.clear()
            for i in new_insts:
                bb.add_instruction(i)
    return n_split


# ---------------------------------------------------------------------------
# the bass program (identical on all 8 cores; per-core data differs)
# ---------------------------------------------------------------------------
def build_decoder():
    nc = bass.Bass(num_devices=N_CORES)

    XT = nc.dram_tensor("XT", [D, S], f16, kind="ExternalInput")
    Wname = {}
    for w in ("Wq1", "Wk1", "Wv1", "Wo1", "Wq2", "Wk2", "Wv2", "Wo2"):
        Wname[w] = nc.dram_tensor(w, [D, D], f16, kind="ExternalInput")
    W1C = nc.dram_tensor("W1C", [F // P, D, P], f16, kind="ExternalInput")
    W2 = nc.dram_tensor("W2", [F, D], f16, kind="ExternalInput")
    B1 = nc.dram_tensor("B1", [P, F // P], f32, kind="ExternalInput")
    B2 = nc.dram_tensor("B2", [P, KT], f32, kind="ExternalInput")
    G = nc.dram_tensor("G", [P, KT], f32, kind="ExternalInput")
    BB = nc.dram_tensor("BB", [P, KT], f32, kind="ExternalInput")
    TRIU = nc.dram_tensor("TRIU", [P, P], f16, kind="ExternalInput")
    OUT = nc.dram_tensor("OUT", [D, SH], f32, kind="ExternalOutput")

    lp = nc.allow_low_precision(reason="f16 operand kernel by design")
    lp.__enter__()
    with TileContext(nc) as tc, ExitStack() as ctx:
        # ---- pools -------------------------------------------------------
        big = ctx.enter_context(tc.tile_pool(name="big", bufs=5))   # 16KB slots
        med = ctx.enter_context(tc.tile_pool(name="med", bufs=4))   # 8KB slots
        hpool = ctx.enter_context(tc.tile_pool(name="hp", bufs=2))
        vpool = ctx.enter_context(tc.tile_pool(name="vp", bufs=1))
        wpool = ctx.enter_context(tc.tile_pool(name="wp", bufs=10))
        epool = ctx.enter_context(tc.tile_pool(name="ep", bufs=5))
        spool = ctx.enter_context(tc.tile_pool(name="sp", bufs=1))
        one = ctx.enter_context(tc.tile_pool(name="one", bufs=1))
        pp = ctx.enter_context(tc.tile_pool(name="pp", bufs=8, space="PSUM"))

        # ---- constants / small inputs -----------------------------------
        triu = one.tile([P, P], f16, tag="triu")
        nc.sync.dma_start(out=triu, in_=TRIU[:, :])
        b1_sb = one.tile([P, F // P], f32, tag="b1")
        nc.sync.dma_start(out=b1_sb, in_=B1[:, :])
        b2_sb = one.tile([P, KT], f32, tag="b2")
        nc.sync.dma_start(out=b2_sb, in_=B2[:, :])
        g_sb = one.tile([P, KT], f32, tag="g")
        nc.sync.dma_start(out=g_sb, in_=G[:, :])
        bb_sb = one.tile([P, KT], f32, tag="bb")
        nc.sync.dma_start(out=bb_sb, in_=BB[:, :])
        ones_col = one.tile([P, 1], f16, tag="onescol")   # colsum lhsT
        nc.vector.memset(ones_col, 1.0)
        ones_row = one.tile([1, P], f16, tag="onesrow")   # bcast lhsT
        nc.vector.memset(ones_row, 1.0)
        eps_t = one.tile([1, 1], f32, tag="eps")
        nc.vector.memset(eps_t, EPS)

        # ---- load X^T ----------------------------------------------------
        xt = big.tile([P, KT, S], f16, tag="big")
        nc.sync.dma_start(out=xt, in_=XT.rearrange("(k p) s -> p k s", p=P))

        # ---- helpers -----------------------------------------------------
        def load_w_slabs(W):
            slabs = []
            for k in range(KT):
                w = wpool.tile([P, D], f16, tag="w")
                nc.sync.dma_start(out=w, in_=W[k * P:(k + 1) * P, :])
                slabs.append(w)
            return slabs

        def project(dst, slabs, rhs_fn, n_chunks):
            """dst[:, m, n*512:+512] = W^T @ rhs ; rhs_fn(k, n) -> [128,512]."""
            for n in range(n_chunks):
                for m in range(KT):
                    ps = pp.tile([P, 512], f32, tag="ps")
                    for k in range(KT):
                        nc.tensor.matmul(
                            ps, slabs[k][:, m * P:(m + 1) * P], rhs_fn(k, n),
                            start=(k == 0), stop=(k == KT - 1))
                    nc.vector.tensor_copy(dst[:, m, n * 512:n * 512 + 512], ps)

        def project_v(dst4, wv_slabs):
            """Seq-major V with interleaved ones columns."""
            nc.vector.memset(dst4[:, :, :, DH:DH + 1], 1.0)
            for si in range(KT):
                for n in range(2):
                    ps = pp.tile([P, 512], f32, tag="ps")
                    for k in range(KT):
                        nc.tensor.matmul(
                            ps, xt[:, k, si * P:(si + 1) * P],
                            wv_slabs[k][:, n * 512:(n + 1) * 512],
                            start=(k == 0), stop=(k == KT - 1))
                    nc.vector.tensor_copy(
                        dst4[:, si, 8 * n:8 * n + 8, 0:DH],
                        ps.rearrange("p (h c) -> p h c", c=DH))

        def attention(qt, kt_sb, v_sb, attnT, n_q, causal):
            """qt [P,KT,n_q], kt_sb [P,KT,S], v_sb [P,KT,H*65] (seq-major,
            ones col), attnT [P,KT,n_q] f16 out."""
            for qb in range(n_q // 512):
                qc = slice(qb * 512, qb * 512 + 512)
                for pr in range(H // 2):          # head pairs share a d-tile
                    m = pr
                    kts = list(range(min(KT, (qb + 1) * 4))) if causal \
                        else list(range(KT))
                    avs = [pp.tile([P, 512], f32, tag="ps", name=f"av{par}") for par in range(2)]
                    for k in kts:
                        c0 = max(0, k * P - qb * 512) if causal else 0
                        ets = []
                        for par in range(2):
                            off = 64 * par
                            sc = pp.tile([P, 512], f32, tag="ps")
                            nc.tensor.matmul(
                                sc[:, c0:512],
                                kt_sb[off:off + 64, m, k * P:(k + 1) * P],
                                qt[off:off + 64, m, qb * 512 + c0:qb * 512 + 512],
                                start=True, stop=True)
                            et = epool.tile([P, 512], f16, tag="et")
                            nc.scalar.activation(
                                out=et[:, c0:512], in_=sc[:, c0:512],
                                func=AF.Exp, scale=0.125)
                            if causal and k >= qb * 4:
                                nc.vector.tensor_mul(
                                    et[:, c0:c0 + P], et[:, c0:c0 + P], triu)
                            ets.append(et)
                        for par in range(2):
                            h16 = 2 * pr + par
                            nc.tensor.matmul(
                                avs[par][0:DH + 1, c0:512],
                                v_sb[:, k, h16 * 65:h16 * 65 + 65],
                                ets[par][:, c0:512],
                                start=(k == kts[0]), stop=(k == kts[-1]))
                    for par in range(2):
                        rd = spool.tile([1, 512], f16, tag="rd", bufs=4)
                        nc.vector.reciprocal(rd, avs[par][DH:DH + 1, :])
                        den = pp.tile([P, 512], f32, tag="ps")
                        nc.tensor.matmul(den[0:64, :], ones_row[0:1, 0:64], rd,
                                         start=True, stop=True)
                        den_sb = spool.tile([64, 512], f32, tag="densb", bufs=2)
                        nc.scalar.copy(den_sb, den[0:64, :])
                        if par == 0:
                            nc.vector.tensor_mul(
                                attnT[0:64, m, qc], avs[par][0:64, :], den_sb)
                        else:
                            avn = spool.tile([64, 512], f16, tag="avn", bufs=3)
                            nc.vector.tensor_mul(avn, avs[par][0:64, :], den_sb)
                            nc.sync.dma_start(out=attnT[64:128, m, qc], in_=avn)

        def layernorm_resid(src, resid, dst, n_cols, final_f32=None):
            """dst = resid + LN(src); LN over the partition (d) axis.
            src/resid/dst are [P, KT, n_cols] f16 tiles (dst may be resid)."""
            n_chunks = n_cols // 512
            stats = []
            for n in range(n_chunks):
                st_a = pp.tile([1, 512], f32, tag="ps")
                st_b = pp.tile([1, 512], f32, tag="ps")
                for m in range(KT):
                    cc = slice(n * 512, n * 512 + 512)
                    sq = spool.tile([P, 512], f16, tag="sq", bufs=3)
                    nc.vector.tensor_mul(sq, src[:, m, cc], src[:, m, cc])
                    nc.tensor.matmul(st_a, ones_col, src[:, m, cc],
                                     start=(m == 0), stop=(m == KT - 1))
                    nc.tensor.matmul(st_b, ones_col, sq,
                                     start=(m == 0), stop=(m == KT - 1))
                stats.append((st_a, st_b))
            for n in range(n_chunks):
                st_a, st_b = stats[n]
                cc = slice(n * 512, n * 512 + 512)
                mu = spool.tile([1, 512], f32, tag="r32", bufs=5)
                nc.scalar.mul(mu, st_a, 1.0 / D)
                msq = spool.tile([1, 512], f32, tag="r32", bufs=5)
                nc.scalar.mul(msq, st_b, 1.0 / D)
                mu2 = spool.tile([1, 512], f32, tag="r32", bufs=5)
                nc.vector.tensor_mul(mu2, mu, mu)
                var = spool.tile([1, 512], f32, tag="r32", bufs=5)
                nc.vector.tensor_sub(var, msq, mu2)
                sd = spool.tile([1, 512], f32, tag="r32", bufs=5)
                nc.scalar.activation(out=sd, in_=var, func=AF.Sqrt,
                                     bias=eps_t, scale=1.0)
                rs32 = spool.tile([1, 512], f32, tag="r32", bufs=5)
                nc.vector.reciprocal(rs32, sd)
                rs16 = spool.tile([1, 512], f16, tag="rd", bufs=4)
                nc.vector.tensor_copy(rs16, rs32)
                qrow = spool.tile([1, 512], f16, tag="rd", bufs=4)
                nc.vector.tensor_mul(qrow, mu, rs32)
                pb_ps = pp.tile([P, 512], f32, tag="ps")
                nc.tensor.matmul(pb_ps, ones_row[0:1, 0:P], rs16,
                                 start=True, stop=True)
                qb_ps = pp.tile([P, 512], f32, tag="ps")
                nc.tensor.matmul(qb_ps, ones_row[0:1, 0:P], qrow,
                                 start=True, stop=True)
                pb16 = spool.tile([P, 512], f16, tag="pb16", bufs=2)
                nc.vector.tensor_copy(pb16, pb_ps)
                qb32 = spool.tile([P, 512], f32, tag="qb32", bufs=2)
                nc.vector.tensor_copy(qb32, qb_ps)
                for m in range(KT):
                    t1 = spool.tile([P, 512], f32, tag="t1", bufs=2)
                    nc.vector.tensor_mul(t1, src[:, m, cc], pb16)
                    nc.vector.tensor_sub(t1, t1, qb32)
                    t3 = spool.tile([P, 512], f16, tag="t3", bufs=3)
                    nc.scalar.activation(out=t3, in_=t1, func=AF.Identity,
                                         bias=bb_sb[:, m:m + 1],
                                         scale=g_sb[:, m:m + 1])
                    if final_f32 is not None:
                        nc.vector.tensor_add(final_f32[:, m, :],
                                             resid[:, m, cc], t3)
                    else:
                        nc.vector.tensor_add(dst[:, m, cc],
                                             resid[:, m, cc], t3)

        # ================= layer 1: causal MHA (full batch) ==============
        qt = big.tile([P, KT, S], f16, tag="big")
        project(qt, load_w_slabs(Wname["Wq1"]),
                lambda k, n: xt[:, k, n * 512:(n + 1) * 512], 2)
        kt_sb = big.tile([P, KT, S], f16, tag="big")
        project(kt_sb, load_w_slabs(Wname["Wk1"]),
                lambda k, n: xt[:, k, n * 512:(n + 1) * 512], 2)
        v_sb = vpool.tile([P, KT, H * (DH + 1)], f16, tag="v")
        project_v(v_sb.rearrange("p k (h c) -> p k h c", c=DH + 1),
                  load_w_slabs(Wname["Wv1"]))

        attnT = big.tile([P, KT, S], f16, tag="big")
        attention(qt, kt_sb, v_sb, attnT, S, causal=True)

        masked = big.tile([P, KT, S], f16, tag="big")
        project(masked, load_w_slabs(Wname["Wo1"]),
                lambda k, n: attnT[:, k, n * 512:(n + 1) * 512], 2)
        layernorm_resid(masked, xt, xt, S)      # xt <- norm_masked

        # ================= layer 2: full MHA (own q-half) ================
        # q-half of this core (0 or 512): dynamic-slice norm_masked on DVE
        qlo_v = (nc.vector.partition_id() % 2) * SH
        nmq = med.tile([P, KT, SH], f16, tag="med")
        xt_flat = xt.rearrange("p k s -> p (k s)")
        for k in range(KT):
            nc.vector.tensor_copy(nmq[:, k, :],
                                  xt_flat[:, bass.ds(qlo_v + k * S, 512)])
        q2t = med.tile([P, KT, SH], f16, tag="med")
        project(q2t, load_w_slabs(Wname["Wq2"]),
                lambda k, n: nmq[:, k, :], 1)
        k2t = big.tile([P, KT, S], f16, tag="big")
        project(k2t, load_w_slabs(Wname["Wk2"]),
                lambda k, n: xt[:, k, n * 512:(n + 1) * 512], 2)
        v2_sb = vpool.tile([P, KT, H * (DH + 1)], f16, tag="v")
        project_v(v2_sb.rearrange("p k (h c) -> p k h c", c=DH + 1),
                  load_w_slabs(Wname["Wv2"]))

        attn2T = med.tile([P, KT, SH], f16, tag="med")
        attention(q2t, k2t, v2_sb, attn2T, SH, causal=False)

        attn2 = med.tile([P, KT, SH], f16, tag="med")
        project(attn2, load_w_slabs(Wname["Wo2"]),
                lambda k, n: attn2T[:, k, 0:512], 1)
        attn2n = med.tile([P, KT, SH], f16, tag="med")
        layernorm_resid(attn2, attn2, attn2n, SH)

        # ================= FFN (own q-half) ==============================
        ff_acc = big.tile([P, KT, SH], f32, tag="big")
        for hc in range(4):                     # 4 hidden chunks of 1024
            h_sb = hpool.tile([P, 8, 512], f16, tag="h")
            for hm in range(8):
                m32 = hc * 8 + hm
                w1t = wpool.tile([P, KT, P], f16, tag="w")
                nc.sync.dma_start(
                    out=w1t, in_=W1C[m32].rearrange("(k p) c -> p k c", p=P))
                ps = pp.tile([P, 512], f32, tag="ps")
                for k in range(KT):
                    nc.tensor.matmul(ps, w1t[:, k, :], attn2n[:, k, 0:512],
                                     start=(k == 0), stop=(k == KT - 1))
                nc.scalar.activation(out=h_sb[:, hm, :], in_=ps, func=AF.Relu,
                                     bias=b1_sb[:, m32:m32 + 1], scale=1.0)
            w2sl = []
            for kk in range(8):
                h32 = hc * 8 + kk
                w2s = wpool.tile([P, D], f16, tag="w")
                nc.sync.dma_start(out=w2s, in_=W2[h32 * P:(h32 + 1) * P, :])
                w2sl.append(w2s)
            for m in range(KT):
                ps = pp.tile([P, 512], f32, tag="ps")
                for kk in range(8):
                    nc.tensor.matmul(ps, w2sl[kk][:, m * P:(m + 1) * P],
                                     h_sb[:, kk, :],
                                     start=(kk == 0), stop=(kk == 7))
                if hc == 0:
                    nc.vector.tensor_copy(ff_acc[:, m, 0:512], ps)
                else:
                    nc.vector.tensor_add(ff_acc[:, m, 0:512],
                                         ff_acc[:, m, 0:512], ps)
        ff = med.tile([P, KT, SH], f16, tag="med")
        for m in range(KT):
            nc.scalar.activation(out=ff[:, m, :], in_=ff_acc[:, m, 0:512],
                                 func=AF.Identity, bias=b2_sb[:, m:m + 1],
                                 scale=1.0)
        final = big.tile([P, KT, SH], f32, tag="big")
        layernorm_resid(ff, attn2n, None, SH, final_f32=final)

        for m in range(KT):
            nc.sync.dma_start(out=OUT[m * P:(m + 1) * P, :], in_=final[:, m, :])

    lp.__exit__(None, None, None)
    _split_multi_waits(nc)
    return nc


# ---------------------------------------------------------------------------
# host wrapper: compile once, shard inputs, run on 8 cores, gather
# ---------------------------------------------------------------------------
_CACHE = {}


def _get_runner():
    if "r" in _CACHE:
        return _CACHE["r"]
    import jax
    from jax.sharding import Mesh, PartitionSpec
    from jax.experimental.shard_map import shard_map

    install_neuronx_cc_hook()
    nc = build_decoder()
    partition_name = nc.partition_id_tensor.name if nc.partition_id_tensor else None
    in_names, out_names, out_avals, zero_outs = [], [], [], []
    for alloc in nc.m.functions[0].allocations:
        if not isinstance(alloc, mybir.MemoryLocationSet):
            continue
        name = alloc.memorylocations[0].name
        if alloc.kind == "ExternalInput":
            if name != partition_name:
                in_names.append(name)
        elif alloc.kind == "ExternalOutput":
            shape = tuple(alloc.tensor_shape)
            dtype = mybir.dt.np(alloc.dtype)
            out_names.append(name)
            out_avals.append(jax.core.ShapedArray(shape, dtype))
            zero_outs.append(np.zeros(shape, dtype))
    all_in_names = list(in_names) + list(out_names)
    if partition_name is not None:
        all_in_names.append(partition_name)

    def _body(*args):
        operands = list(args)
        if partition_name is not None:
            operands.append(partition_id_tensor())
        outs = _bass_exec_p.bind(
            *operands,
            out_avals=tuple(out_avals),
            in_names=tuple(all_in_names),
            out_names=tuple(out_names),
            lowering_input_output_aliases=(),
            sim_require_finite=True,
            sim_require_nnan=True,
            nc=nc,
        )
        return tuple(outs)

    devices = jax.devices()[:N_CORES]
    mesh = Mesh(np.asarray(devices), ("core",))
    n_in = len(in_names) + len(zero_outs)
    fn = jax.jit(
        shard_map(_body, mesh=mesh,
                  in_specs=(PartitionSpec("core"),) * n_in,
                  out_specs=(PartitionSpec("core"),) * len(out_names),
                  check_rep=False),
        keep_unused=True,
    )
    _CACHE["r"] = (fn, mesh, in_names, out_names, out_avals, zero_outs)
    return _CACHE["r"]


def _prep_in_maps(X, Wq1, Wk1, Wv1, Wo1, Wq2, Wk2, Wv2, Wo2, ln_g, ln_b,
                  W1, b1, W2, b2):
    h16 = lambda a: np.ascontiguousarray(np.asarray(a), dtype=np.float16)
    f32c = lambda a: np.ascontiguousarray(np.asarray(a), dtype=np.float32)
    shared = {
        "Wq1": h16(Wq1), "Wk1": h16(Wk1), "Wv1": h16(Wv1), "Wo1": h16(Wo1),
        "Wq2": h16(Wq2), "Wk2": h16(Wk2), "Wv2": h16(Wv2), "Wo2": h16(Wo2),
        "W1C": h16(np.asarray(W1).reshape(D, F // P, P).transpose(1, 0, 2)),
        "W2": h16(W2),
        "B1": f32c(np.asarray(b1).reshape(F // P, P).T),
        "B2": f32c(np.asarray(b2).reshape(KT, P).T),
        "G": f32c(np.asarray(ln_g).reshape(KT, P).T),
        "BB": f32c(np.asarray(ln_b).reshape(KT, P).T),
        "TRIU": np.triu(np.ones((P, P), np.float16)),
    }
    Xn = np.asarray(X)
    in_maps = []
    for c in range(N_CORES):
        m = dict(shared)
        m["XT"] = h16(Xn[c // 2].T)
        in_maps.append(m)
    return in_maps


def kernel(**inputs) -> np.ndarray:
    import jax

    fn, mesh, in_names, out_names, out_avals, zero_outs = _get_runner()
    in_maps = _prep_in_maps(**inputs)
    concat_in = [
        np.concatenate([in_maps[c][name] for c in range(N_CORES)], axis=0)
        for name in in_names
    ]
    concat_zeros = [
        np.zeros((N_CORES * z.shape[0], *z.shape[1:]), z.dtype)
        for z in zero_outs
    ]
    outs = fn(*concat_in, *concat_zeros)
    jax.block_until_ready(outs)
    i_out = out_names.index("OUT")
    per_core = np.asarray(outs[i_out]).reshape(N_CORES, D, SH)
    # assemble: core c -> batch c//2, query half c%2 ; output is [B, S, D]
    result = np.empty((B, S, D), np.float32)
    for c in range(N_CORES):
        b, h = c // 2, c % 2
        result[b, h * SH:(h + 1) * SH, :] = per_core[c].T
    return result


# revision 19
# speedup vs baseline: 2.2524x; 2.2524x over previous
"""Trainium2 Bass kernel for nn_DecoderBlock (dense transformer block).

Sharding: 8 NeuronCores = 4 batch elements x 2 sequence halves.
Each core computes layer-1 (causal MHA + LN + residual) for its whole batch
element (duplicated across the pair - avoids any collective), then layer-2
full attention + FFN only for its own 512-query half, selected with a
partition-id-driven dynamic slice.  Activations are kept feature-major
[d, s] so every projection is a plain (weights-stationary) matmul; softmax
runs on transposed scores [k, q] with denominators obtained for free from an
extra ones-column in V; LayerNorm stats (over the partition axis) come from
ones-vector matmuls on the PE.

All matmul operands are f16 (fp32 accumulation in PSUM); LN / softmax scalar
math stays fp32.  Expected end-to-end relative error vs the fp32 reference:
~1e-3.
"""
import sys
sys.path.insert(0, '/opt/trn_rl_repo')
import numpy as np
from contextlib import ExitStack

import concourse.bass as bass
import concourse.mybir as mybir
from concourse.tile import TileContext
from concourse.bass2jax import _bass_exec_p, partition_id_tensor, install_neuronx_cc_hook

f32 = mybir.dt.float32
f16 = mybir.dt.float16
AF = mybir.ActivationFunctionType

D = 1024          # d_model
S = 1024          # sequence length
B = 4             # batch
H = 16            # heads
DH = 64           # head dim
F = 4096          # ffn hidden
P = 128
KT = D // P       # 8 d-tiles
SH = 512          # per-core sequence half
EPS = 1e-5
N_CORES = 8


# ---------------------------------------------------------------------------
# walrus workaround: split multi-sem-wait instructions into single-wait NOPs
# ---------------------------------------------------------------------------
def _split_multi_waits(nc, max_waits=1):
    n_split = 0
    for fn in nc.m.functions:
        for bb in fn.blocks:
            new_insts = []
            for inst in bb.instructions:
                si = inst.sync_info
                waits = list(si.on_wait) if si is not None else []
                if len(waits) > max_waits:
                    keep = waits[-max_waits:]
                    for w in waits[:-max_waits]:
                        nop = mybir.InstNoOp(
                            name=nc.get_next_instruction_name(),
                            engine=inst.engine,
                            sync_info=mybir.SyncInfo(on_wait=[w], on_update=[]),
                            bass_nofuse=True,
                        )
                        nc.register_instruction(nop)
                        new_insts.append(nop)
                    inst.sync_info = mybir.SyncInfo(
                        on_wait=keep, on_update=list(si.on_update))
                    n_split += 1
                new_insts.append(inst)
            bb.instructions.clear()
            for i in new_insts:
                bb.add_instruction(i)
    return n_split


# ---------------------------------------------------------------------------
# the bass program (identical on all 8 cores; per-core data differs)
# ---------------------------------------------------------------------------
def build_decoder():
    nc = bass.Bass(num_devices=N_CORES)

    XT = nc.dram_tensor("XT", [D, S], f16, kind="ExternalInput")
    Wname = {}
    for w in ("Wq1", "Wk1", "Wv1", "Wo1", "Wq2", "Wk2", "Wv2", "Wo2"):
        Wname[w] = nc.dram_tensor(w, [D, D], f16, kind="ExternalInput")
    W1C = nc.dram_tensor("W1C", [F // P, D, P], f16, kind="ExternalInput")
    W2 = nc.dram_tensor("W2", [F, D], f16, kind="ExternalInput")
    B1 = nc.dram_tensor("B1", [P, F // P], f32, kind="ExternalInput")
    B2 = nc.dram_tensor("B2", [P, KT], f32, kind="ExternalInput")
    G = nc.dram_tensor("G", [P, KT], f32, kind="ExternalInput")
    BB = nc.dram_tensor("BB", [P, KT], f32, kind="ExternalInput")
    TRIU = nc.dram_tensor("TRIU", [P, P], f16, kind="ExternalInput")
    OUT = nc.dram_tensor("OUT", [D, SH], f32, kind="ExternalOutput")

    lp = nc.allow_low_precision(reason="f16 operand kernel by design")
    lp.__enter__()
    with TileContext(nc) as tc, ExitStack() as ctx:
        # ---- pools -------------------------------------------------------
        big = ctx.enter_context(tc.tile_pool(name="big", bufs=5))   # 16KB slots
        med = ctx.enter_context(tc.tile_pool(name="med", bufs=4))   # 8KB slots
        hpool = ctx.enter_context(tc.tile_pool(name="hp", bufs=2))
        vpool = ctx.enter_context(tc.tile_pool(name="vp", bufs=1))
        wpool = ctx.enter_context(tc.tile_pool(name="wp", bufs=10))
        epool = ctx.enter_context(tc.tile_pool(name="ep", bufs=5))
        spool = ctx.enter_context(tc.tile_pool(name="sp", bufs=1))
        one = ctx.enter_context(tc.tile_pool(name="one", bufs=1))
        pp = ctx.enter_context(tc.tile_pool(name="pp", bufs=8, space="PSUM"))

        # ---- constants / small inputs -----------------------------------
        triu = one.tile([P, P], f16, tag="triu")
        nc.sync.dma_start(out=triu, in_=TRIU[:, :])
        b1_sb = one.tile([P, F // P], f32, tag="b1")
        nc.sync.dma_start(out=b1_sb, in_=B1[:, :])
        b2_sb = one.tile([P, KT], f32, tag="b2")
        nc.sync.dma_start(out=b2_sb, in_=B2[:, :])
        g_sb = one.tile([P, KT], f32, tag="g")
        nc.sync.dma_start(out=g_sb, in_=G[:, :])
        bb_sb = one.tile([P, KT], f32, tag="bb")
        nc.sync.dma_start(out=bb_sb, in_=BB[:, :])
        ones_col = one.tile([P, 1], f16, tag="onescol")   # colsum lhsT
        nc.vector.memset(ones_col, 1.0)
        ones_row = one.tile([1, P], f16, tag="onesrow")   # bcast lhsT
        nc.vector.memset(ones_row, 1.0)
        eps_t = one.tile([1, 1], f32, tag="eps")
        nc.vector.memset(eps_t, EPS)
        eb1_t = one.tile([P, 1], f32, tag="eb1")
        nc.vector.memset(eb1_t, -2.0)
        eb2_t = one.tile([P, 1], f32, tag="eb2")
        nc.vector.memset(eb2_t, -8.0)

        # ---- load X^T ----------------------------------------------------
        xt = big.tile([P, KT, S], f16, tag="big")
        nc.sync.dma_start(out=xt, in_=XT.rearrange("(k p) s -> p k s", p=P))

        # ---- helpers -----------------------------------------------------
        def load_w_slabs(W):
            slabs = []
            for k in range(KT):
                w = wpool.tile([P, D], f16, tag="w")
                nc.sync.dma_start(out=w, in_=W[k * P:(k + 1) * P, :])
                slabs.append(w)
            return slabs

        def project(dst, slabs, rhs_fn, n_chunks):
            """dst[:, m, n*512:+512] = W^T @ rhs ; rhs_fn(k, n) -> [128,512]."""
            for n in range(n_chunks):
                for m in range(KT):
                    ps = pp.tile([P, 512], f32, tag="ps")
                    for k in range(KT):
                        nc.tensor.matmul(
                            ps, slabs[k][:, m * P:(m + 1) * P], rhs_fn(k, n),
                            start=(k == 0), stop=(k == KT - 1))
                    nc.vector.tensor_copy(dst[:, m, n * 512:n * 512 + 512], ps)

        def project_v(dst4, wv_slabs):
            """Seq-major V with interleaved ones columns."""
            nc.vector.memset(dst4[:, :, :, DH:DH + 1], 1.0)
            for si in range(KT):
                for n in range(2):
                    ps = pp.tile([P, 512], f32, tag="ps")
                    for k in range(KT):
                        nc.tensor.matmul(
                            ps, xt[:, k, si * P:(si + 1) * P],
                            wv_slabs[k][:, n * 512:(n + 1) * 512],
                            start=(k == 0), stop=(k == KT - 1))
                    nc.vector.tensor_copy(
                        dst4[:, si, 8 * n:8 * n + 8, 0:DH],
                        ps.rearrange("p (h c) -> p h c", c=DH))

        def attention(qt, kt_sb, v_sb, attnT, n_q, causal, exp_bias=None):
            """qt [P,KT,n_q], kt_sb [P,KT,S], v_sb [P,KT,H*65] (seq-major,
            ones col), attnT [P,KT,n_q] f16 out."""
            for qb in range(n_q // 512):
                qc = slice(qb * 512, qb * 512 + 512)
                for pr in range(H // 2):          # head pairs share a d-tile
                    m = pr
                    kts = list(range(min(KT, (qb + 1) * 4))) if causal \
                        else list(range(KT))
                    avs = [pp.tile([P, 512], f32, tag="ps", name=f"av{par}") for par in range(2)]
                    for k in kts:
                        c0 = max(0, k * P - qb * 512) if causal else 0
                        ets = []
                        for par in range(2):
                            off = 64 * par
                            sc = pp.tile([P, 512], f32, tag="ps")
                            nc.tensor.matmul(
                                sc[:, c0:512],
                                kt_sb[off:off + 64, m, k * P:(k + 1) * P],
                                qt[off:off + 64, m, qb * 512 + c0:qb * 512 + 512],
                                start=True, stop=True)
                            et = epool.tile([P, 512], f16, tag="et")
                            nc.scalar.activation(
                                out=et[:, c0:512], in_=sc[:, c0:512],
                                func=AF.Exp, scale=0.125, bias=exp_bias)
                            if causal and k >= qb * 4:
                                nc.vector.tensor_mul(
                                    et[:, c0:c0 + P], et[:, c0:c0 + P], triu)
                            ets.append(et)
                        for par in range(2):
                            h16 = 2 * pr + par
                            nc.tensor.matmul(
                                avs[par][0:DH + 1, c0:512],
                                v_sb[:, k, h16 * 65:h16 * 65 + 65],
                                ets[par][:, c0:512],
                                start=(k == kts[0]), stop=(k == kts[-1]))
                    for par in range(2):
                        rd = spool.tile([1, 512], f16, tag="rd", bufs=4)
                        nc.vector.reciprocal(rd, avs[par][DH:DH + 1, :])
                        den = pp.tile([P, 512], f32, tag="ps")
                        nc.tensor.matmul(den[0:64, :], ones_row[0:1, 0:64], rd,
                                         start=True, stop=True)
                        den_sb = spool.tile([64, 512], f32, tag="densb", bufs=2)
                        nc.scalar.copy(den_sb, den[0:64, :])
                        if par == 0:
                            nc.vector.tensor_mul(
                                attnT[0:64, m, qc], avs[par][0:64, :], den_sb)
                        else:
                            avn = spool.tile([64, 512], f16, tag="avn", bufs=3)
                            nc.vector.tensor_mul(avn, avs[par][0:64, :], den_sb)
                            nc.sync.dma_start(out=attnT[64:128, m, qc], in_=avn)

        def layernorm_resid(src, resid, dst, n_cols, final_f32=None):
            """dst = resid + LN(src); LN over the partition (d) axis.
            src/resid/dst are [P, KT, n_cols] f16 tiles (dst may be resid)."""
            n_chunks = n_cols // 512
            stats = []
            for n in range(n_chunks):
                st_a = pp.tile([1, 512], f32, tag="ps")
                st_b = pp.tile([1, 512], f32, tag="ps")
                for m in range(KT):
                    cc = slice(n * 512, n * 512 + 512)
                    sq = spool.tile([P, 512], f16, tag="sq", bufs=3)
                    nc.vector.tensor_mul(sq, src[:, m, cc], src[:, m, cc])
                    nc.tensor.matmul(st_a, ones_col, src[:, m, cc],
                                     start=(m == 0), stop=(m == KT - 1))
                    nc.tensor.matmul(st_b, ones_col, sq,
                                     start=(m == 0), stop=(m == KT - 1))
                stats.append((st_a, st_b))
            for n in range(n_chunks):
                st_a, st_b = stats[n]
                cc = slice(n * 512, n * 512 + 512)
                mu = spool.tile([1, 512], f32, tag="r32", bufs=5)
                nc.scalar.mul(mu, st_a, 1.0 / D)
                msq = spool.tile([1, 512], f32, tag="r32", bufs=5)
                nc.scalar.mul(msq, st_b, 1.0 / D)
                mu2 = spool.tile([1, 512], f32, tag="r32", bufs=5)
                nc.vector.tensor_mul(mu2, mu, mu)
                var = spool.tile([1, 512], f32, tag="r32", bufs=5)
                nc.vector.tensor_sub(var, msq, mu2)
                sd = spool.tile([1, 512], f32, tag="r32", bufs=5)
                nc.scalar.activation(out=sd, in_=var, func=AF.Sqrt,
                                     bias=eps_t, scale=1.0)
                rs32 = spool.tile([1, 512], f32, tag="r32", bufs=5)
                nc.vector.reciprocal(rs32, sd)
                rs16 = spool.tile([1, 512], f16, tag="rd", bufs=4)
                nc.vector.tensor_copy(rs16, rs32)
                qrow = spool.tile([1, 512], f16, tag="rd", bufs=4)
                nc.vector.tensor_mul(qrow, mu, rs32)
                pb_ps = pp.tile([P, 512], f32, tag="ps")
                nc.tensor.matmul(pb_ps, ones_row[0:1, 0:P], rs16,
                                 start=True, stop=True)
                qb_ps = pp.tile([P, 512], f32, tag="ps")
                nc.tensor.matmul(qb_ps, ones_row[0:1, 0:P], qrow,
                                 start=True, stop=True)
                pb16 = spool.tile([P, 512], f16, tag="pb16", bufs=2)
                nc.vector.tensor_copy(pb16, pb_ps)
                qb32 = spool.tile([P, 512], f32, tag="qb32", bufs=2)
                nc.vector.tensor_copy(qb32, qb_ps)
                for m in range(KT):
                    t1 = spool.tile([P, 512], f32, tag="t1", bufs=2)
                    nc.vector.tensor_mul(t1, src[:, m, cc], pb16)
                    nc.vector.tensor_sub(t1, t1, qb32)
                    t3 = spool.tile([P, 512], f16, tag="t3", bufs=3)
                    nc.scalar.activation(out=t3, in_=t1, func=AF.Identity,
                                         bias=bb_sb[:, m:m + 1],
                                         scale=g_sb[:, m:m + 1])
                    if final_f32 is not None:
                        nc.vector.tensor_add(final_f32[:, m, :],
                                             resid[:, m, cc], t3)
                    else:
                        nc.vector.tensor_add(dst[:, m, cc],
                                             resid[:, m, cc], t3)

        # ================= layer 1: causal MHA (full batch) ==============
        qt = big.tile([P, KT, S], f16, tag="big")
        project(qt, load_w_slabs(Wname["Wq1"]),
                lambda k, n: xt[:, k, n * 512:(n + 1) * 512], 2)
        kt_sb = big.tile([P, KT, S], f16, tag="big")
        project(kt_sb, load_w_slabs(Wname["Wk1"]),
                lambda k, n: xt[:, k, n * 512:(n + 1) * 512], 2)
        v_sb = vpool.tile([P, KT, H * (DH + 1)], f16, tag="v")
        project_v(v_sb.rearrange("p k (h c) -> p k h c", c=DH + 1),
                  load_w_slabs(Wname["Wv1"]))

        attnT = big.tile([P, KT, S], f16, tag="big")
        attention(qt, kt_sb, v_sb, attnT, S, causal=True, exp_bias=eb1_t)

        masked = big.tile([P, KT, S], f16, tag="big")
        project(masked, load_w_slabs(Wname["Wo1"]),
                lambda k, n: attnT[:, k, n * 512:(n + 1) * 512], 2)
        layernorm_resid(masked, xt, xt, S)      # xt <- norm_masked

        # ================= layer 2: full MHA (own q-half) ================
        # q-half of this core (0 or 512): dynamic-slice norm_masked on DVE
        qlo_v = (nc.vector.partition_id() % 2) * SH
        nmq = med.tile([P, KT, SH], f16, tag="med")
        xt_flat = xt.rearrange("p k s -> p (k s)")
        for k in range(KT):
            nc.vector.tensor_copy(nmq[:, k, :],
                                  xt_flat[:, bass.ds(qlo_v + k * S, 512)])
        q2t = med.tile([P, KT, SH], f16, tag="med")
        project(q2t, load_w_slabs(Wname["Wq2"]),
                lambda k, n: nmq[:, k, :], 1)
        k2t = big.tile([P, KT, S], f16, tag="big")
        project(k2t, load_w_slabs(Wname["Wk2"]),
                lambda k, n: xt[:, k, n * 512:(n + 1) * 512], 2)
        v2_sb = vpool.tile([P, KT, H * (DH + 1)], f16, tag="v")
        project_v(v2_sb.rearrange("p k (h c) -> p k h c", c=DH + 1),
                  load_w_slabs(Wname["Wv2"]))

        attn2T = med.tile([P, KT, SH], f16, tag="med")
        attention(q2t, k2t, v2_sb, attn2T, SH, causal=False, exp_bias=eb2_t)

        attn2 = med.tile([P, KT, SH], f16, tag="med")
        project(attn2, load_w_slabs(Wname["Wo2"]),
                lambda k, n: attn2T[:, k, 0:512], 1)
        attn2n = med.tile([P, KT, SH], f16, tag="med")
        layernorm_resid(attn2, attn2, attn2n, SH)

        # ================= FFN (own q-half) ==============================
        ff_acc = big.tile([P, KT, SH], f32, tag="big")
        for hc in range(4):                     # 4 hidden chunks of 1024
            h_sb = hpool.tile([P, 8, 512], f16, tag="h")
            for hm in range(8):
                m32 = hc * 8 + hm
                w1t = wpool.tile([P, KT, P], f16, tag="w")
                nc.sync.dma_start(
                    out=w1t, in_=W1C[m32].rearrange("(k p) c -> p k c", p=P))
                ps = pp.tile([P, 512], f32, tag="ps")
                for k in range(KT):
                    nc.tensor.matmul(ps, w1t[:, k, :], attn2n[:, k, 0:512],
                                     start=(k == 0), stop=(k == KT - 1))
                nc.scalar.activation(out=h_sb[:, hm, :], in_=ps, func=AF.Relu,
                                     bias=b1_sb[:, m32:m32 + 1], scale=1.0)
            w2sl = []
            for kk in range(8):
                h32 = hc * 8 + kk
                w2s = wpool.tile([P, D], f16, tag="w")
                nc.sync.dma_start(out=w2s, in_=W2[h32 * P:(h32 + 1) * P, :])
                w2sl.append(w2s)
            for m in range(KT):
                ps = pp.tile([P, 512], f32, tag="ps")
                for kk in range(8):
                    nc.tensor.matmul(ps, w2sl[kk][:, m * P:(m + 1) * P],
                                     h_sb[:, kk, :],
                                     start=(kk == 0), stop=(kk == 7))
                if hc == 0:
                    nc.vector.tensor_copy(ff_acc[:, m, 0:512], ps)
                else:
                    nc.vector.tensor_add(ff_acc[:, m, 0:512],
                                         ff_acc[:, m, 0:512], ps)
        ff = med.tile([P, KT, SH], f16, tag="med")
        for m in range(KT):
            nc.scalar.activation(out=ff[:, m, :], in_=ff_acc[:, m, 0:512],
                                 func=AF.Identity, bias=b2_sb[:, m:m + 1],
                                 scale=1.0)
        final = big.tile([P, KT, SH], f32, tag="big")
        layernorm_resid(ff, attn2n, None, SH, final_f32=final)

        for m in range(KT):
            nc.sync.dma_start(out=OUT[m * P:(m + 1) * P, :], in_=final[:, m, :])

    lp.__exit__(None, None, None)
    _split_multi_waits(nc)
    return nc


# ---------------------------------------------------------------------------
# host wrapper: compile once, shard inputs, run on 8 cores, gather
# ---------------------------------------------------------------------------
_CACHE = {}


def _get_runner():
    if "r" in _CACHE:
        return _CACHE["r"]
    import jax
    from jax.sharding import Mesh, PartitionSpec
    from jax.experimental.shard_map import shard_map

    install_neuronx_cc_hook()
    nc = build_decoder()
    partition_name = nc.partition_id_tensor.name if nc.partition_id_tensor else None
    in_names, out_names, out_avals, zero_outs = [], [], [], []
    for alloc in nc.m.functions[0].allocations:
        if not isinstance(alloc, mybir.MemoryLocationSet):
            continue
        name = alloc.memorylocations[0].name
        if alloc.kind == "ExternalInput":
            if name != partition_name:
                in_names.append(name)
        elif alloc.kind == "ExternalOutput":
            shape = tuple(alloc.tensor_shape)
            dtype = mybir.dt.np(alloc.dtype)
            out_names.append(name)
            out_avals.append(jax.core.ShapedArray(shape, dtype))
            zero_outs.append(np.zeros(shape, dtype))
    all_in_names = list(in_names) + list(out_names)
    if partition_name is not None:
        all_in_names.append(partition_name)

    def _body(*args):
        operands = list(args)
        if partition_name is not None:
            operands.append(partition_id_tensor())
        outs = _bass_exec_p.bind(
            *operands,
            out_avals=tuple(out_avals),
            in_names=tuple(all_in_names),
            out_names=tuple(out_names),
            lowering_input_output_aliases=(),
            sim_require_finite=True,
            sim_require_nnan=True,
            nc=nc,
        )
        return tuple(outs)

    devices = jax.devices()[:N_CORES]
    mesh = Mesh(np.asarray(devices), ("core",))
    n_in = len(in_names) + len(zero_outs)
    fn = jax.jit(
        shard_map(_body, mesh=mesh,
                  in_specs=(PartitionSpec("core"),) * n_in,
                  out_specs=(PartitionSpec("core"),) * len(out_names),
                  check_rep=False),
        keep_unused=True,
    )
    _CACHE["r"] = (fn, mesh, in_names, out_names, out_avals, zero_outs)
    return _CACHE["r"]


def _prep_in_maps(X, Wq1, Wk1, Wv1, Wo1, Wq2, Wk2, Wv2, Wo2, ln_g, ln_b,
                  W1, b1, W2, b2):
    h16 = lambda a: np.ascontiguousarray(np.asarray(a), dtype=np.float16)
    f32c = lambda a: np.ascontiguousarray(np.asarray(a), dtype=np.float32)
    shared = {
        "Wq1": h16(Wq1), "Wk1": h16(Wk1), "Wv1": h16(Wv1), "Wo1": h16(Wo1),
        "Wq2": h16(Wq2), "Wk2": h16(Wk2), "Wv2": h16(Wv2), "Wo2": h16(Wo2),
        "W1C": h16(np.asarray(W1).reshape(D, F // P, P).transpose(1, 0, 2)),
        "W2": h16(W2),
        "B1": f32c(np.asarray(b1).reshape(F // P, P).T),
        "B2": f32c(np.asarray(b2).reshape(KT, P).T),
        "G": f32c(np.asarray(ln_g).reshape(KT, P).T),
        "BB": f32c(np.asarray(ln_b).reshape(KT, P).T),
        "TRIU": np.triu(np.ones((P, P), np.float16)),
    }
    Xn = np.asarray(X)
    in_maps = []
    for c in range(N_CORES):
        m = dict(shared)
        m["XT"] = h16(Xn[c // 2].T)
        in_maps.append(m)
    return in_maps


def kernel(**inputs) -> np.ndarray:
    import jax

    fn, mesh, in_names, out_names, out_avals, zero_outs = _get_runner()
    in_maps = _prep_in_maps(**inputs)
    concat_in = [
        np.concatenate([in_maps[c][name] for c in range(N_CORES)], axis=0)
        for name in in_names
    ]
    concat_zeros = [
        np.zeros((N_CORES * z.shape[0], *z.shape[1:]), z.dtype)
        for z in zero_outs
    ]
    outs = fn(*concat_in, *concat_zeros)
    jax.block_until_ready(outs)
    i_out = out_names.index("OUT")
    per_core = np.asarray(outs[i_out]).reshape(N_CORES, D, SH)
    # assemble: core c -> batch c//2, query half c%2 ; output is [B, S, D]
    result = np.empty((B, S, D), np.float32)
    for c in range(N_CORES):
        b, h = c // 2, c % 2
        result[b, h * SH:(h + 1) * SH, :] = per_core[c].T
    return result


# revision 20
# speedup vs baseline: 10658.4683x; 4732.1043x over previous
"""Trainium2 Bass kernel for nn_DecoderBlock (dense transformer block).

Sharding: 8 NeuronCores = 4 batch elements x 2 sequence halves.
Each core computes layer-1 (causal MHA + LN + residual) for its whole batch
element (duplicated across the pair - avoids any collective), then layer-2
full attention + FFN only for its own 512-query half, selected with a
partition-id-driven dynamic slice.  Activations are kept feature-major
[d, s] so every projection is a plain (weights-stationary) matmul; softmax
runs on transposed scores [k, q] with denominators obtained for free from an
extra ones-column in V; LayerNorm stats (over the partition axis) come from
ones-vector matmuls on the PE.

All matmul operands are f16 (fp32 accumulation in PSUM); LN / softmax scalar
math stays fp32.  Expected end-to-end relative error vs the fp32 reference:
~1e-3.
"""
import sys
sys.path.insert(0, '/opt/trn_rl_repo')
import numpy as np
from contextlib import ExitStack

import concourse.bass as bass
import concourse.mybir as mybir
from concourse.tile import TileContext
from concourse.bass2jax import _bass_exec_p, partition_id_tensor, install_neuronx_cc_hook

f32 = mybir.dt.float32
f16 = mybir.dt.float16
AF = mybir.ActivationFunctionType

D = 1024          # d_model
S = 1024          # sequence length
B = 4             # batch
H = 16            # heads
DH = 64           # head dim
F = 4096          # ffn hidden
P = 128
KT = D // P       # 8 d-tiles
SH = 512          # per-core sequence half
EPS = 1e-5
N_CORES = 8


# ---------------------------------------------------------------------------
# walrus workaround: split multi-sem-wait instructions into single-wait NOPs
# ---------------------------------------------------------------------------
def _split_multi_waits(nc, max_waits=1):
    n_split = 0
    for fn in nc.m.functions:
        for bb in fn.blocks:
            new_insts = []
            for inst in bb.instructions:
                si = inst.sync_info
                waits = list(si.on_wait) if si is not None else []
                if len(waits) > max_waits:
                    keep = waits[-max_waits:]
                    for w in waits[:-max_waits]:
                        nop = mybir.InstNoOp(
                            name=nc.get_next_instruction_name(),
                            engine=inst.engine,
                            sync_info=mybir.SyncInfo(on_wait=[w], on_update=[]),
                            bass_nofuse=True,
                        )
                        nc.register_instruction(nop)
                        new_insts.append(nop)
                    inst.sync_info = mybir.SyncInfo(
                        on_wait=keep, on_update=list(si.on_update))
                    n_split += 1
                new_insts.append(inst)
            bb.instructions.clear()
            for i in new_insts:
                bb.add_instruction(i)
    return n_split


# ---------------------------------------------------------------------------
# the bass program (identical on all 8 cores; per-core data differs)
# ---------------------------------------------------------------------------
def build_decoder(reps=0):
    nc = bass.Bass(num_devices=N_CORES)

    XT = nc.dram_tensor("XT", [D, S], f16, kind="ExternalInput")
    Wname = {}
    for w in ("Wq1", "Wk1", "Wv1", "Wo1", "Wq2", "Wk2", "Wv2", "Wo2"):
        Wname[w] = nc.dram_tensor(w, [D, D], f16, kind="ExternalInput")
    W1C = nc.dram_tensor("W1C", [F // P, D, P], f16, kind="ExternalInput")
    W2 = nc.dram_tensor("W2", [F, D], f16, kind="ExternalInput")
    B1 = nc.dram_tensor("B1", [P, F // P], f32, kind="ExternalInput")
    B2 = nc.dram_tensor("B2", [P, KT], f32, kind="ExternalInput")
    G = nc.dram_tensor("G", [P, KT], f32, kind="ExternalInput")
    BB = nc.dram_tensor("BB", [P, KT], f32, kind="ExternalInput")
    TRIU = nc.dram_tensor("TRIU", [P, P], f16, kind="ExternalInput")
    OUT = nc.dram_tensor("OUT", [D, SH], f32, kind="ExternalOutput")

    lp = nc.allow_low_precision(reason="f16 operand kernel by design")
    lp.__enter__()
    with TileContext(nc) as tc, ExitStack() as ctx:
        # ---- pools -------------------------------------------------------
        big = ctx.enter_context(tc.tile_pool(name="big", bufs=5))   # 16KB slots
        med = ctx.enter_context(tc.tile_pool(name="med", bufs=4))   # 8KB slots
        hpool = ctx.enter_context(tc.tile_pool(name="hp", bufs=2))
        vpool = ctx.enter_context(tc.tile_pool(name="vp", bufs=1))
        wpool = ctx.enter_context(tc.tile_pool(name="wp", bufs=10))
        epool = ctx.enter_context(tc.tile_pool(name="ep", bufs=5))
        spool = ctx.enter_context(tc.tile_pool(name="sp", bufs=1))
        one = ctx.enter_context(tc.tile_pool(name="one", bufs=1))
        pp = ctx.enter_context(tc.tile_pool(name="pp", bufs=8, space="PSUM"))

        # ---- constants / small inputs -----------------------------------
        triu = one.tile([P, P], f16, tag="triu")
        nc.sync.dma_start(out=triu, in_=TRIU[:, :])
        b1_sb = one.tile([P, F // P], f32, tag="b1")
        nc.sync.dma_start(out=b1_sb, in_=B1[:, :])
        b2_sb = one.tile([P, KT], f32, tag="b2")
        nc.sync.dma_start(out=b2_sb, in_=B2[:, :])
        g_sb = one.tile([P, KT], f32, tag="g")
        nc.sync.dma_start(out=g_sb, in_=G[:, :])
        bb_sb = one.tile([P, KT], f32, tag="bb")
        nc.sync.dma_start(out=bb_sb, in_=BB[:, :])
        ones_col = one.tile([P, 1], f16, tag="onescol")   # colsum lhsT
        nc.vector.memset(ones_col, 1.0)
        ones_row = one.tile([1, P], f16, tag="onesrow")   # bcast lhsT
        nc.vector.memset(ones_row, 1.0)
        eps_t = one.tile([1, 1], f32, tag="eps")
        nc.vector.memset(eps_t, EPS)
        eb1_t = one.tile([P, 1], f32, tag="eb1")
        nc.vector.memset(eb1_t, -2.0)
        eb2_t = one.tile([P, 1], f32, tag="eb2")
        nc.vector.memset(eb2_t, -8.0)

        rep_cm = tc.For_i(0, reps, 1) if reps else None
        if rep_cm is not None:
            rep_cm.__enter__()

        # ---- load X^T ----------------------------------------------------
        xt = big.tile([P, KT, S], f16, tag="big")
        nc.sync.dma_start(out=xt, in_=XT.rearrange("(k p) s -> p k s", p=P))

        # ---- helpers -----------------------------------------------------
        def load_w_slabs(W):
            slabs = []
            for k in range(KT):
                w = wpool.tile([P, D], f16, tag="w")
                nc.sync.dma_start(out=w, in_=W[k * P:(k + 1) * P, :])
                slabs.append(w)
            return slabs

        def project(dst, slabs, rhs_fn, n_chunks):
            """dst[:, m, n*512:+512] = W^T @ rhs ; rhs_fn(k, n) -> [128,512]."""
            for n in range(n_chunks):
                for m in range(KT):
                    ps = pp.tile([P, 512], f32, tag="ps")
                    for k in range(KT):
                        nc.tensor.matmul(
                            ps, slabs[k][:, m * P:(m + 1) * P], rhs_fn(k, n),
                            start=(k == 0), stop=(k == KT - 1))
                    nc.vector.tensor_copy(dst[:, m, n * 512:n * 512 + 512], ps)

        def project_v(dst4, wv_slabs):
            """Seq-major V with interleaved ones columns."""
            nc.vector.memset(dst4[:, :, :, DH:DH + 1], 1.0)
            for si in range(KT):
                for n in range(2):
                    ps = pp.tile([P, 512], f32, tag="ps")
                    for k in range(KT):
                        nc.tensor.matmul(
                            ps, xt[:, k, si * P:(si + 1) * P],
                            wv_slabs[k][:, n * 512:(n + 1) * 512],
                            start=(k == 0), stop=(k == KT - 1))
                    nc.vector.tensor_copy(
                        dst4[:, si, 8 * n:8 * n + 8, 0:DH],
                        ps.rearrange("p (h c) -> p h c", c=DH))

        def attention(qt, kt_sb, v_sb, attnT, n_q, causal, exp_bias=None):
            """qt [P,KT,n_q], kt_sb [P,KT,S], v_sb [P,KT,H*65] (seq-major,
            ones col), attnT [P,KT,n_q] f16 out."""
            for qb in range(n_q // 512):
                qc = slice(qb * 512, qb * 512 + 512)
                for pr in range(H // 2):          # head pairs share a d-tile
                    m = pr
                    kts = list(range(min(KT, (qb + 1) * 4))) if causal \
                        else list(range(KT))
                    avs = [pp.tile([P, 512], f32, tag="ps", name=f"av{par}") for par in range(2)]
                    for k in kts:
                        c0 = max(0, k * P - qb * 512) if causal else 0
                        ets = []
                        for par in range(2):
                            off = 64 * par
                            sc = pp.tile([P, 512], f32, tag="ps")
                            nc.tensor.matmul(
                                sc[:, c0:512],
                                kt_sb[off:off + 64, m, k * P:(k + 1) * P],
                                qt[off:off + 64, m, qb * 512 + c0:qb * 512 + 512],
                                start=True, stop=True)
                            et = epool.tile([P, 512], f16, tag="et")
                            nc.scalar.activation(
                                out=et[:, c0:512], in_=sc[:, c0:512],
                                func=AF.Exp, scale=0.125, bias=exp_bias)
                            if causal and k >= qb * 4:
                                nc.vector.tensor_mul(
                                    et[:, c0:c0 + P], et[:, c0:c0 + P], triu)
                            ets.append(et)
                        for par in range(2):
                            h16 = 2 * pr + par
                            nc.tensor.matmul(
                                avs[par][0:DH + 1, c0:512],
                                v_sb[:, k, h16 * 65:h16 * 65 + 65],
                                ets[par][:, c0:512],
                                start=(k == kts[0]), stop=(k == kts[-1]))
                    for par in range(2):
                        rd = spool.tile([1, 512], f16, tag="rd", bufs=4)
                        nc.vector.reciprocal(rd, avs[par][DH:DH + 1, :])
                        den = pp.tile([P, 512], f32, tag="ps")
                        nc.tensor.matmul(den[0:64, :], ones_row[0:1, 0:64], rd,
                                         start=True, stop=True)
                        den_sb = spool.tile([64, 512], f32, tag="densb", bufs=2)
                        nc.scalar.copy(den_sb, den[0:64, :])
                        if par == 0:
                            nc.vector.tensor_mul(
                                attnT[0:64, m, qc], avs[par][0:64, :], den_sb)
                        else:
                            avn = spool.tile([64, 512], f16, tag="avn", bufs=3)
                            nc.vector.tensor_mul(avn, avs[par][0:64, :], den_sb)
                            nc.sync.dma_start(out=attnT[64:128, m, qc], in_=avn)

        def layernorm_resid(src, resid, dst, n_cols, final_f32=None):
            """dst = resid + LN(src); LN over the partition (d) axis.
            src/resid/dst are [P, KT, n_cols] f16 tiles (dst may be resid)."""
            n_chunks = n_cols // 512
            stats = []
            for n in range(n_chunks):
                st_a = pp.tile([1, 512], f32, tag="ps")
                st_b = pp.tile([1, 512], f32, tag="ps")
                for m in range(KT):
                    cc = slice(n * 512, n * 512 + 512)
                    sq = spool.tile([P, 512], f16, tag="sq", bufs=3)
                    nc.vector.tensor_mul(sq, src[:, m, cc], src[:, m, cc])
                    nc.tensor.matmul(st_a, ones_col, src[:, m, cc],
                                     start=(m == 0), stop=(m == KT - 1))
                    nc.tensor.matmul(st_b, ones_col, sq,
                                     start=(m == 0), stop=(m == KT - 1))
                stats.append((st_a, st_b))
            for n in range(n_chunks):
                st_a, st_b = stats[n]
                cc = slice(n * 512, n * 512 + 512)
                mu = spool.tile([1, 512], f32, tag="r32", bufs=5)
                nc.scalar.mul(mu, st_a, 1.0 / D)
                msq = spool.tile([1, 512], f32, tag="r32", bufs=5)
                nc.scalar.mul(msq, st_b, 1.0 / D)
                mu2 = spool.tile([1, 512], f32, tag="r32", bufs=5)
                nc.vector.tensor_mul(mu2, mu, mu)
                var = spool.tile([1, 512], f32, tag="r32", bufs=5)
                nc.vector.tensor_sub(var, msq, mu2)
                sd = spool.tile([1, 512], f32, tag="r32", bufs=5)
                nc.scalar.activation(out=sd, in_=var, func=AF.Sqrt,
                                     bias=eps_t, scale=1.0)
                rs32 = spool.tile([1, 512], f32, tag="r32", bufs=5)
                nc.vector.reciprocal(rs32, sd)
                rs16 = spool.tile([1, 512], f16, tag="rd", bufs=4)
                nc.vector.tensor_copy(rs16, rs32)
                qrow = spool.tile([1, 512], f16, tag="rd", bufs=4)
                nc.vector.tensor_mul(qrow, mu, rs32)
                pb_ps = pp.tile([P, 512], f32, tag="ps")
                nc.tensor.matmul(pb_ps, ones_row[0:1, 0:P], rs16,
                                 start=True, stop=True)
                qb_ps = pp.tile([P, 512], f32, tag="ps")
                nc.tensor.matmul(qb_ps, ones_row[0:1, 0:P], qrow,
                                 start=True, stop=True)
                pb16 = spool.tile([P, 512], f16, tag="pb16", bufs=2)
                nc.vector.tensor_copy(pb16, pb_ps)
                qb32 = spool.tile([P, 512], f32, tag="qb32", bufs=2)
                nc.vector.tensor_copy(qb32, qb_ps)
                for m in range(KT):
                    t1 = spool.tile([P, 512], f32, tag="t1", bufs=2)
                    nc.vector.tensor_mul(t1, src[:, m, cc], pb16)
                    nc.vector.tensor_sub(t1, t1, qb32)
                    t3 = spool.tile([P, 512], f16, tag="t3", bufs=3)
                    nc.scalar.activation(out=t3, in_=t1, func=AF.Identity,
                                         bias=bb_sb[:, m:m + 1],
                                         scale=g_sb[:, m:m + 1])
                    if final_f32 is not None:
                        nc.vector.tensor_add(final_f32[:, m, :],
                                             resid[:, m, cc], t3)
                    else:
                        nc.vector.tensor_add(dst[:, m, cc],
                                             resid[:, m, cc], t3)

        # ================= layer 1: causal MHA (full batch) ==============
        qt = big.tile([P, KT, S], f16, tag="big")
        project(qt, load_w_slabs(Wname["Wq1"]),
                lambda k, n: xt[:, k, n * 512:(n + 1) * 512], 2)
        kt_sb = big.tile([P, KT, S], f16, tag="big")
        project(kt_sb, load_w_slabs(Wname["Wk1"]),
                lambda k, n: xt[:, k, n * 512:(n + 1) * 512], 2)
        v_sb = vpool.tile([P, KT, H * (DH + 1)], f16, tag="v")
        project_v(v_sb.rearrange("p k (h c) -> p k h c", c=DH + 1),
                  load_w_slabs(Wname["Wv1"]))

        attnT = big.tile([P, KT, S], f16, tag="big")
        attention(qt, kt_sb, v_sb, attnT, S, causal=True, exp_bias=eb1_t)

        masked = big.tile([P, KT, S], f16, tag="big")
        project(masked, load_w_slabs(Wname["Wo1"]),
                lambda k, n: attnT[:, k, n * 512:(n + 1) * 512], 2)
        layernorm_resid(masked, xt, xt, S)      # xt <- norm_masked

        # ================= layer 2: full MHA (own q-half) ================
        # q-half of this core (0 or 512): dynamic-slice norm_masked on DVE
        qlo_v = (nc.vector.partition_id() % 2) * SH
        nmq = med.tile([P, KT, SH], f16, tag="med")
        xt_flat = xt.rearrange("p k s -> p (k s)")
        for k in range(KT):
            nc.vector.tensor_copy(nmq[:, k, :],
                                  xt_flat[:, bass.ds(qlo_v + k * S, 512)])
        q2t = med.tile([P, KT, SH], f16, tag="med")
        project(q2t, load_w_slabs(Wname["Wq2"]),
                lambda k, n: nmq[:, k, :], 1)
        k2t = big.tile([P, KT, S], f16, tag="big")
        project(k2t, load_w_slabs(Wname["Wk2"]),
                lambda k, n: xt[:, k, n * 512:(n + 1) * 512], 2)
        v2_sb = vpool.tile([P, KT, H * (DH + 1)], f16, tag="v")
        project_v(v2_sb.rearrange("p k (h c) -> p k h c", c=DH + 1),
                  load_w_slabs(Wname["Wv2"]))

        attn2T = med.tile([P, KT, SH], f16, tag="med")
        attention(q2t, k2t, v2_sb, attn2T, SH, causal=False, exp_bias=eb2_t)

        attn2 = med.tile([P, KT, SH], f16, tag="med")
        project(attn2, load_w_slabs(Wname["Wo2"]),
                lambda k, n: attn2T[:, k, 0:512], 1)
        attn2n = med.tile([P, KT, SH], f16, tag="med")
        layernorm_resid(attn2, attn2, attn2n, SH)

        # ================= FFN (own q-half) ==============================
        ff_acc = big.tile([P, KT, SH], f32, tag="big")
        for hc in range(4):                     # 4 hidden chunks of 1024
            h_sb = hpool.tile([P, 8, 512], f16, tag="h")
            for hm in range(8):
                m32 = hc * 8 + hm
                w1t = wpool.tile([P, KT, P], f16, tag="w")
                nc.sync.dma_start(
                    out=w1t, in_=W1C[m32].rearrange("(k p) c -> p k c", p=P))
                ps = pp.tile([P, 512], f32, tag="ps")
                for k in range(KT):
                    nc.tensor.matmul(ps, w1t[:, k, :], attn2n[:, k, 0:512],
                                     start=(k == 0), stop=(k == KT - 1))
                nc.scalar.activation(out=h_sb[:, hm, :], in_=ps, func=AF.Relu,
                                     bias=b1_sb[:, m32:m32 + 1], scale=1.0)
            w2sl = []
            for kk in range(8):
                h32 = hc * 8 + kk
                w2s = wpool.tile([P, D], f16, tag="w")
                nc.sync.dma_start(out=w2s, in_=W2[h32 * P:(h32 + 1) * P, :])
                w2sl.append(w2s)
            for m in range(KT):
                ps = pp.tile([P, 512], f32, tag="ps")
                for kk in range(8):
                    nc.tensor.matmul(ps, w2sl[kk][:, m * P:(m + 1) * P],
                                     h_sb[:, kk, :],
                                     start=(kk == 0), stop=(kk == 7))
                if hc == 0:
                    nc.vector.tensor_copy(ff_acc[:, m, 0:512], ps)
                else:
                    nc.vector.tensor_add(ff_acc[:, m, 0:512],
                                         ff_acc[:, m, 0:512], ps)
        ff = med.tile([P, KT, SH], f16, tag="med")
        for m in range(KT):
            nc.scalar.activation(out=ff[:, m, :], in_=ff_acc[:, m, 0:512],
                                 func=AF.Identity, bias=b2_sb[:, m:m + 1],
                                 scale=1.0)
        final = big.tile([P, KT, SH], f32, tag="big")
        layernorm_resid(ff, attn2n, None, SH, final_f32=final)

        for m in range(KT):
            nc.sync.dma_start(out=OUT[m * P:(m + 1) * P, :], in_=final[:, m, :])

        if rep_cm is not None:
            rep_cm.__exit__(None, None, None)

    lp.__exit__(None, None, None)
    _split_multi_waits(nc)
    return nc


# ---------------------------------------------------------------------------
# host wrapper: compile once, shard inputs, run on 8 cores, gather
# ---------------------------------------------------------------------------
_CACHE = {}


def _get_runner():
    if "r" in _CACHE:
        return _CACHE["r"]
    import jax
    from jax.sharding import Mesh, PartitionSpec
    from jax.experimental.shard_map import shard_map

    install_neuronx_cc_hook()
    nc = build_decoder()
    partition_name = nc.partition_id_tensor.name if nc.partition_id_tensor else None
    in_names, out_names, out_avals, zero_outs = [], [], [], []
    for alloc in nc.m.functions[0].allocations:
        if not isinstance(alloc, mybir.MemoryLocationSet):
            continue
        name = alloc.memorylocations[0].name
        if alloc.kind == "ExternalInput":
            if name != partition_name:
                in_names.append(name)
        elif alloc.kind == "ExternalOutput":
            shape = tuple(alloc.tensor_shape)
            dtype = mybir.dt.np(alloc.dtype)
            out_names.append(name)
            out_avals.append(jax.core.ShapedArray(shape, dtype))
            zero_outs.append(np.zeros(shape, dtype))
    all_in_names = list(in_names) + list(out_names)
    if partition_name is not None:
        all_in_names.append(partition_name)

    def _body(*args):
        operands = list(args)
        if partition_name is not None:
            operands.append(partition_id_tensor())
        outs = _bass_exec_p.bind(
            *operands,
            out_avals=tuple(out_avals),
            in_names=tuple(all_in_names),
            out_names=tuple(out_names),
            lowering_input_output_aliases=(),
            sim_require_finite=True,
            sim_require_nnan=True,
            nc=nc,
        )
        return tuple(outs)

    devices = jax.devices()[:N_CORES]
    mesh = Mesh(np.asarray(devices), ("core",))
    n_in = len(in_names) + len(zero_outs)
    fn = jax.jit(
        shard_map(_body, mesh=mesh,
                  in_specs=(PartitionSpec("core"),) * n_in,
                  out_specs=(PartitionSpec("core"),) * len(out_names),
                  check_rep=False),
        keep_unused=True,
    )
    _CACHE["r"] = (fn, mesh, in_names, out_names, out_avals, zero_outs)
    return _CACHE["r"]


def _prep_in_maps(X, Wq1, Wk1, Wv1, Wo1, Wq2, Wk2, Wv2, Wo2, ln_g, ln_b,
                  W1, b1, W2, b2):
    h16 = lambda a: np.ascontiguousarray(np.asarray(a), dtype=np.float16)
    f32c = lambda a: np.ascontiguousarray(np.asarray(a), dtype=np.float32)
    shared = {
        "Wq1": h16(Wq1), "Wk1": h16(Wk1), "Wv1": h16(Wv1), "Wo1": h16(Wo1),
        "Wq2": h16(Wq2), "Wk2": h16(Wk2), "Wv2": h16(Wv2), "Wo2": h16(Wo2),
        "W1C": h16(np.asarray(W1).reshape(D, F // P, P).transpose(1, 0, 2)),
        "W2": h16(W2),
        "B1": f32c(np.asarray(b1).reshape(F // P, P).T),
        "B2": f32c(np.asarray(b2).reshape(KT, P).T),
        "G": f32c(np.asarray(ln_g).reshape(KT, P).T),
        "BB": f32c(np.asarray(ln_b).reshape(KT, P).T),
        "TRIU": np.triu(np.ones((P, P), np.float16)),
    }
    Xn = np.asarray(X)
    in_maps = []
    for c in range(N_CORES):
        m = dict(shared)
        m["XT"] = h16(Xn[c // 2].T)
        in_maps.append(m)
    return in_maps


def kernel(**inputs) -> np.ndarray:
    import jax

    fn, mesh, in_names, out_names, out_avals, zero_outs = _get_runner()
    in_maps = _prep_in_maps(**inputs)
    concat_in = [
        np.concatenate([in_maps[c][name] for c in range(N_CORES)], axis=0)
        for name in in_names
    ]
    concat_zeros = [
        np.zeros((N_CORES * z.shape[0], *z.shape[1:]), z.dtype)
        for z in zero_outs
    ]
    outs = fn(*concat_in, *concat_zeros)
    jax.block_until_ready(outs)
    i_out = out_names.index("OUT")
    per_core = np.asarray(outs[i_out]).reshape(N_CORES, D, SH)
    # assemble: core c -> batch c//2, query half c%2 ; output is [B, S, D]
    result = np.empty((B, S, D), np.float32)
    for c in range(N_CORES):
        b, h = c // 2, c % 2
        result[b, h * SH:(h + 1) * SH, :] = per_core[c].T
    return result


# revision 25
# speedup vs baseline: 13395.7976x; 1.2568x over previous
"""Trainium2 Bass kernel for nn_DecoderBlock (dense transformer block).

Sharding: 8 NeuronCores = 4 batch elements x 2 sequence halves.
Each core computes layer-1 (causal MHA + LN + residual) for its whole batch
element (duplicated across the pair - avoids any collective), then layer-2
full attention + FFN only for its own 512-query half, selected with a
partition-id-driven dynamic slice.  Activations are kept feature-major
[d, s] so every projection is a plain (weights-stationary) matmul; softmax
runs on transposed scores [k, q] with denominators obtained for free from an
extra ones-column in V; LayerNorm stats (over the partition axis) come from
ones-vector matmuls on the PE.

All matmul operands are f16 (fp32 accumulation in PSUM); LN / softmax scalar
math stays fp32.  Expected end-to-end relative error vs the fp32 reference:
~1e-3.
"""
import sys
sys.path.insert(0, '/opt/trn_rl_repo')
import numpy as np
from contextlib import ExitStack

import concourse.bass as bass
import concourse.mybir as mybir
from concourse.tile import TileContext
from concourse.bass2jax import _bass_exec_p, partition_id_tensor, install_neuronx_cc_hook

f32 = mybir.dt.float32
f16 = mybir.dt.float16
AF = mybir.ActivationFunctionType

D = 1024          # d_model
S = 1024          # sequence length
B = 4             # batch
H = 16            # heads
DH = 64           # head dim
F = 4096          # ffn hidden
P = 128
KT = D // P       # 8 d-tiles
SH = 512          # per-core sequence half
EPS = 1e-5
N_CORES = 8


# ---------------------------------------------------------------------------
# walrus workaround: split multi-sem-wait instructions into single-wait NOPs
# ---------------------------------------------------------------------------
def _split_multi_waits(nc, max_waits=1):
    n_split = 0
    for fn in nc.m.functions:
        for bb in fn.blocks:
            new_insts = []
            for inst in bb.instructions:
                si = inst.sync_info
                waits = list(si.on_wait) if si is not None else []
                if len(waits) > max_waits:
                    keep = waits[-max_waits:]
                    for w in waits[:-max_waits]:
                        nop = mybir.InstNoOp(
                            name=nc.get_next_instruction_name(),
                            engine=inst.engine,
                            sync_info=mybir.SyncInfo(on_wait=[w], on_update=[]),
                            bass_nofuse=True,
                        )
                        nc.register_instruction(nop)
                        new_insts.append(nop)
                    inst.sync_info = mybir.SyncInfo(
                        on_wait=keep, on_update=list(si.on_update))
                    n_split += 1
                new_insts.append(inst)
            bb.instructions.clear()
            for i in new_insts:
                bb.add_instruction(i)
    return n_split


# ---------------------------------------------------------------------------
# the bass program (identical on all 8 cores; per-core data differs)
# ---------------------------------------------------------------------------
def build_decoder(reps=0):
    # reps<0: runtime rep count from NREPS input
    nc = bass.Bass(num_devices=N_CORES)

    XT = nc.dram_tensor("XT", [D, S], f16, kind="ExternalInput")
    Wname = {}
    for w in ("Wq1", "Wk1", "Wv1", "Wo1", "Wq2", "Wk2", "Wv2", "Wo2"):
        Wname[w] = nc.dram_tensor(w, [D, D], f16, kind="ExternalInput")
    W1C = nc.dram_tensor("W1C", [F // P, D, P], f16, kind="ExternalInput")
    W2 = nc.dram_tensor("W2", [F, D], f16, kind="ExternalInput")
    B1 = nc.dram_tensor("B1", [P, F // P], f32, kind="ExternalInput")
    B2 = nc.dram_tensor("B2", [P, KT], f32, kind="ExternalInput")
    G = nc.dram_tensor("G", [P, KT], f32, kind="ExternalInput")
    BB = nc.dram_tensor("BB", [P, KT], f32, kind="ExternalInput")
    TRIU = nc.dram_tensor("TRIU", [P, P], f16, kind="ExternalInput")
    OUT = nc.dram_tensor("OUT", [D, SH], f32, kind="ExternalOutput")

    lp = nc.allow_low_precision(reason="f16 operand kernel by design")
    lp.__enter__()
    with TileContext(nc) as tc, ExitStack() as ctx:
        # ---- pools -------------------------------------------------------
        big = ctx.enter_context(tc.tile_pool(name="big", bufs=5))   # 16KB slots
        med = ctx.enter_context(tc.tile_pool(name="med", bufs=4))   # 8KB slots
        hpool = ctx.enter_context(tc.tile_pool(name="hp", bufs=2))
        vpool = ctx.enter_context(tc.tile_pool(name="vp", bufs=1))
        wpool = ctx.enter_context(tc.tile_pool(name="wp", bufs=10))
        epool = ctx.enter_context(tc.tile_pool(name="ep", bufs=4))
        spool = ctx.enter_context(tc.tile_pool(name="sp", bufs=1))
        one = ctx.enter_context(tc.tile_pool(name="one", bufs=1))
        pp = ctx.enter_context(tc.tile_pool(name="pp", bufs=4, space="PSUM"))

        # ---- constants / small inputs -----------------------------------
        triu = one.tile([P, P], f16, tag="triu")
        nc.sync.dma_start(out=triu, in_=TRIU[:, :])
        b1_sb = one.tile([P, F // P], f32, tag="b1")
        nc.sync.dma_start(out=b1_sb, in_=B1[:, :])
        b2_sb = one.tile([P, KT], f32, tag="b2")
        nc.sync.dma_start(out=b2_sb, in_=B2[:, :])
        g_sb = one.tile([P, KT], f32, tag="g")
        nc.sync.dma_start(out=g_sb, in_=G[:, :])
        bb_sb = one.tile([P, KT], f32, tag="bb")
        nc.sync.dma_start(out=bb_sb, in_=BB[:, :])
        ones_col = one.tile([P, 1], f16, tag="onescol")   # colsum lhsT
        nc.vector.memset(ones_col, 1.0)
        ones_row = one.tile([1, P], f16, tag="onesrow")   # bcast lhsT
        nc.vector.memset(ones_row, 1.0)
        eps_t = one.tile([1, 1], f32, tag="eps")
        nc.vector.memset(eps_t, EPS)
        eb1_t = one.tile([P, 1], f32, tag="eb1")
        nc.vector.memset(eb1_t, -2.0)
        eb2_t = one.tile([P, 1], f32, tag="eb2")
        nc.vector.memset(eb2_t, -8.0)

        rep_cm = tc.For_i(0, reps, 1) if reps else None
        if rep_cm is not None:
            rep_cm.__enter__()

        # ---- load X^T ----------------------------------------------------
        xt = big.tile([P, KT, S], f16, tag="big")
        nc.sync.dma_start(out=xt, in_=XT.rearrange("(k p) s -> p k s", p=P))

        # ---- helpers -----------------------------------------------------
        def load_w_slabs(W):
            slabs = []
            for k in range(KT):
                w = wpool.tile([P, D], f16, tag="w")
                nc.sync.dma_start(out=w, in_=W[k * P:(k + 1) * P, :])
                slabs.append(w)
            return slabs

        def project(dst, slabs, rhs_fn, n_chunks):
            """dst[:, m, n*512:+512] = W^T @ rhs ; rhs_fn(k, n) -> [128,512]."""
            for n in range(n_chunks):
                for m in range(KT):
                    ps = pp.tile([P, 512], f32, tag="ps")
                    for k in range(KT):
                        nc.tensor.matmul(
                            ps, slabs[k][:, m * P:(m + 1) * P], rhs_fn(k, n),
                            start=(k == 0), stop=(k == KT - 1))
                    nc.vector.tensor_copy(dst[:, m, n * 512:n * 512 + 512], ps)

        def project_v(dst4, wv_slabs):
            """Seq-major V with interleaved ones columns."""
            nc.vector.memset(dst4[:, :, :, DH:DH + 1], 1.0)
            for si in range(KT):
                for n in range(2):
                    ps = pp.tile([P, 512], f32, tag="ps")
                    for k in range(KT):
                        nc.tensor.matmul(
                            ps, xt[:, k, si * P:(si + 1) * P],
                            wv_slabs[k][:, n * 512:(n + 1) * 512],
                            start=(k == 0), stop=(k == KT - 1))
                    nc.vector.tensor_copy(
                        dst4[:, si, 8 * n:8 * n + 8, 0:DH],
                        ps.rearrange("p (h c) -> p h c", c=DH))

        def attention(qt, kt_sb, v_sb, attnT, n_q, causal, exp_bias=None):
            """qt [P,KT,n_q], kt_sb [P,KT,S], v_sb [P,KT,H*65] (seq-major,
            ones col), attnT [P,KT,n_q] f16 out.  Head pairs share a d-tile;
            their score tiles live in one 2-bank psum tile so the exp runs as
            a single [128, 1024] ACT op."""
            for qb in range(n_q // 512):
                qc = slice(qb * 512, qb * 512 + 512)
                for pr in range(H // 2):
                    m = pr
                    kts = list(range(min(KT, (qb + 1) * 4))) if causal \
                        else list(range(KT))
                    avs = [pp.tile([P, 512], f32, tag="ps", name=f"av{par}") for par in range(2)]
                    for k in kts:
                        c0 = max(0, k * P - qb * 512) if causal else 0
                        sc = pp.tile([P, 1024], f32, tag="ps2", bufs=2)
                        for par in range(2):
                            off = 64 * par
                            nc.tensor.matmul(
                                sc[:, 512 * par + c0:512 * par + 512],
                                kt_sb[off:off + 64, m, k * P:(k + 1) * P],
                                qt[off:off + 64, m, qb * 512 + c0:qb * 512 + 512],
                                start=True, stop=True)
                        et = epool.tile([P, 1024], f16, tag="et")
                        if c0 == 0:
                            nc.scalar.activation(
                                out=et, in_=sc, func=AF.Exp, scale=0.125,
                                bias=exp_bias)
                        else:
                            for par in range(2):
                                nc.scalar.activation(
                                    out=et[:, 512 * par + c0:512 * par + 512],
                                    in_=sc[:, 512 * par + c0:512 * par + 512],
                                    func=AF.Exp, scale=0.125, bias=exp_bias)
                        if causal and k >= qb * 4:
                            for par in range(2):
                                nc.vector.tensor_mul(
                                    et[:, 512 * par + c0:512 * par + c0 + P],
                                    et[:, 512 * par + c0:512 * par + c0 + P],
                                    triu)
                        for par in range(2):
                            h16 = 2 * pr + par
                            nc.tensor.matmul(
                                avs[par][0:DH + 1, c0:512],
                                v_sb[:, k, h16 * 65:h16 * 65 + 65],
                                et[:, 512 * par + c0:512 * par + 512],
                                start=(k == kts[0]), stop=(k == kts[-1]))
                    for par in range(2):
                        rd = spool.tile([1, 512], f16, tag="rd", bufs=4)
                        nc.vector.reciprocal(rd, avs[par][DH:DH + 1, :])
                        den = pp.tile([P, 512], f32, tag="ps")
                        nc.tensor.matmul(den[0:64, :], ones_row[0:1, 0:64], rd,
                                         start=True, stop=True)
                        den_sb = spool.tile([64, 512], f32, tag="densb", bufs=2)
                        nc.vector.tensor_copy(den_sb, den[0:64, :])
                        if par == 0:
                            nc.vector.tensor_mul(
                                attnT[0:64, m, qc], avs[par][0:64, :], den_sb)
                        else:
                            avn = spool.tile([64, 512], f16, tag="avn", bufs=3)
                            nc.vector.tensor_mul(avn, avs[par][0:64, :], den_sb)
                            nc.sync.dma_start(out=attnT[64:128, m, qc], in_=avn)

        def layernorm_resid(src, resid, dst, n_cols, final_f32=None):
            """dst = resid + LN(src); LN over the partition (d) axis.
            src/resid/dst are [P, KT, n_cols] f16 tiles (dst may be resid)."""
            n_chunks = n_cols // 512
            stats = []
            for n in range(n_chunks):
                st_a = pp.tile([1, 512], f32, tag="ps")
                st_b = pp.tile([1, 512], f32, tag="ps")
                for m in range(KT):
                    cc = slice(n * 512, n * 512 + 512)
                    sq = spool.tile([P, 512], f16, tag="sq", bufs=2)
                    nc.vector.tensor_mul(sq, src[:, m, cc], src[:, m, cc])
                    nc.tensor.matmul(st_a, ones_col, src[:, m, cc],
                                     start=(m == 0), stop=(m == KT - 1))
                    nc.tensor.matmul(st_b, ones_col, sq,
                                     start=(m == 0), stop=(m == KT - 1))
                stats.append((st_a, st_b))
            for n in range(n_chunks):
                st_a, st_b = stats[n]
                cc = slice(n * 512, n * 512 + 512)
                mu = spool.tile([1, 512], f32, tag="r32", bufs=4)
                nc.scalar.mul(mu, st_a, 1.0 / D)
                msq = spool.tile([1, 512], f32, tag="r32", bufs=4)
                nc.scalar.mul(msq, st_b, 1.0 / D)
                mu2 = spool.tile([1, 512], f32, tag="r32", bufs=4)
                nc.vector.tensor_mul(mu2, mu, mu)
                var = spool.tile([1, 512], f32, tag="r32", bufs=4)
                nc.vector.tensor_sub(var, msq, mu2)
                sd = spool.tile([1, 512], f32, tag="r32", bufs=4)
                nc.scalar.activation(out=sd, in_=var, func=AF.Sqrt,
                                     bias=eps_t, scale=1.0)
                rs32 = spool.tile([1, 512], f32, tag="r32", bufs=4)
                nc.vector.reciprocal(rs32, sd)
                rs16 = spool.tile([1, 512], f16, tag="rd", bufs=4)
                nc.vector.tensor_copy(rs16, rs32)
                qrow = spool.tile([1, 512], f16, tag="rd", bufs=4)
                nc.vector.tensor_mul(qrow, mu, rs32)
                pb_ps = pp.tile([P, 512], f32, tag="ps")
                nc.tensor.matmul(pb_ps, ones_row[0:1, 0:P], rs16,
                                 start=True, stop=True)
                qb_ps = pp.tile([P, 512], f32, tag="ps")
                nc.tensor.matmul(qb_ps, ones_row[0:1, 0:P], qrow,
                                 start=True, stop=True)
                pb16 = spool.tile([P, 512], f16, tag="pb16", bufs=2)
                nc.vector.tensor_copy(pb16, pb_ps)
                qb32 = spool.tile([P, 512], f32, tag="qb32", bufs=2)
                nc.vector.tensor_copy(qb32, qb_ps)
                for m in range(KT):
                    t1 = spool.tile([P, 512], f32, tag="t1", bufs=2)
                    nc.vector.tensor_mul(t1, src[:, m, cc], pb16)
                    nc.vector.tensor_sub(t1, t1, qb32)
                    t3 = spool.tile([P, 512], f16, tag="t3", bufs=3)
                    nc.scalar.activation(out=t3, in_=t1, func=AF.Identity,
                                         bias=bb_sb[:, m:m + 1],
                                         scale=g_sb[:, m:m + 1])
                    if final_f32 is not None:
                        nc.vector.tensor_add(final_f32[:, m, :],
                                             resid[:, m, cc], t3)
                    else:
                        nc.vector.tensor_add(dst[:, m, cc],
                                             resid[:, m, cc], t3)

        # ================= layer 1: causal MHA (full batch) ==============
        qt = big.tile([P, KT, S], f16, tag="big")
        project(qt, load_w_slabs(Wname["Wq1"]),
                lambda k, n: xt[:, k, n * 512:(n + 1) * 512], 2)
        kt_sb = big.tile([P, KT, S], f16, tag="big")
        project(kt_sb, load_w_slabs(Wname["Wk1"]),
                lambda k, n: xt[:, k, n * 512:(n + 1) * 512], 2)
        v_sb = vpool.tile([P, KT, H * (DH + 1)], f16, tag="v")
        project_v(v_sb.rearrange("p k (h c) -> p k h c", c=DH + 1),
                  load_w_slabs(Wname["Wv1"]))

        attnT = big.tile([P, KT, S], f16, tag="big")
        attention(qt, kt_sb, v_sb, attnT, S, causal=True, exp_bias=eb1_t)

        masked = big.tile([P, KT, S], f16, tag="big")
        project(masked, load_w_slabs(Wname["Wo1"]),
                lambda k, n: attnT[:, k, n * 512:(n + 1) * 512], 2)
        layernorm_resid(masked, xt, xt, S)      # xt <- norm_masked

        # ================= layer 2: full MHA (own q-half) ================
        # q-half of this core (0 or 512): dynamic-slice norm_masked on DVE
        qlo_v = (nc.vector.partition_id() % 2) * SH
        nmq = med.tile([P, KT, SH], f16, tag="med")
        xt_flat = xt.rearrange("p k s -> p (k s)")
        for k in range(KT):
            nc.vector.tensor_copy(nmq[:, k, :],
                                  xt_flat[:, bass.ds(qlo_v + k * S, 512)])
        q2t = med.tile([P, KT, SH], f16, tag="med")
        project(q2t, load_w_slabs(Wname["Wq2"]),
                lambda k, n: nmq[:, k, :], 1)
        k2t = big.tile([P, KT, S], f16, tag="big")
        project(k2t, load_w_slabs(Wname["Wk2"]),
                lambda k, n: xt[:, k, n * 512:(n + 1) * 512], 2)
        v2_sb = vpool.tile([P, KT, H * (DH + 1)], f16, tag="v")
        project_v(v2_sb.rearrange("p k (h c) -> p k h c", c=DH + 1),
                  load_w_slabs(Wname["Wv2"]))

        attn2T = med.tile([P, KT, SH], f16, tag="med")
        attention(q2t, k2t, v2_sb, attn2T, SH, causal=False, exp_bias=eb2_t)

        attn2 = med.tile([P, KT, SH], f16, tag="med")
        project(attn2, load_w_slabs(Wname["Wo2"]),
                lambda k, n: attn2T[:, k, 0:512], 1)
        attn2n = med.tile([P, KT, SH], f16, tag="med")
        layernorm_resid(attn2, attn2, attn2n, SH)

        # ================= FFN (own q-half) ==============================
        ff_acc = big.tile([P, KT, SH], f32, tag="big")
        for hc in range(4):                     # 4 hidden chunks of 1024
            h_sb = hpool.tile([P, 8, 512], f16, tag="h")
            for hm in range(8):
                m32 = hc * 8 + hm
                w1t = wpool.tile([P, KT, P], f16, tag="w")
                nc.sync.dma_start(
                    out=w1t, in_=W1C[m32].rearrange("(k p) c -> p k c", p=P))
                ps = pp.tile([P, 512], f32, tag="ps")
                for k in range(KT):
                    nc.tensor.matmul(ps, w1t[:, k, :], attn2n[:, k, 0:512],
                                     start=(k == 0), stop=(k == KT - 1))
                nc.scalar.activation(out=h_sb[:, hm, :], in_=ps, func=AF.Relu,
                                     bias=b1_sb[:, m32:m32 + 1], scale=1.0)
            w2sl = []
            for kk in range(8):
                h32 = hc * 8 + kk
                w2s = wpool.tile([P, D], f16, tag="w")
                nc.sync.dma_start(out=w2s, in_=W2[h32 * P:(h32 + 1) * P, :])
                w2sl.append(w2s)
            for m in range(KT):
                ps = pp.tile([P, 512], f32, tag="ps")
                for kk in range(8):
                    nc.tensor.matmul(ps, w2sl[kk][:, m * P:(m + 1) * P],
                                     h_sb[:, kk, :],
                                     start=(kk == 0), stop=(kk == 7))
                if hc == 0:
                    nc.vector.tensor_copy(ff_acc[:, m, 0:512], ps)
                else:
                    nc.vector.tensor_add(ff_acc[:, m, 0:512],
                                         ff_acc[:, m, 0:512], ps)
        ff = med.tile([P, KT, SH], f16, tag="med")
        for m in range(KT):
            nc.scalar.activation(out=ff[:, m, :], in_=ff_acc[:, m, 0:512],
                                 func=AF.Identity, bias=b2_sb[:, m:m + 1],
                                 scale=1.0)
        final = big.tile([P, KT, SH], f32, tag="big")
        layernorm_resid(ff, attn2n, None, SH, final_f32=final)

        for m in range(KT):
            nc.sync.dma_start(out=OUT[m * P:(m + 1) * P, :], in_=final[:, m, :])

        if rep_cm is not None:
            rep_cm.__exit__(None, None, None)

    lp.__exit__(None, None, None)
    _split_multi_waits(nc)
    return nc


# ---------------------------------------------------------------------------
# host wrapper: compile once, shard inputs, run on 8 cores, gather
# ---------------------------------------------------------------------------
_CACHE = {}


def _get_runner():
    if "r" in _CACHE:
        return _CACHE["r"]
    import jax
    from jax.sharding import Mesh, PartitionSpec
    from jax.experimental.shard_map import shard_map

    install_neuronx_cc_hook()
    nc = build_decoder()
    partition_name = nc.partition_id_tensor.name if nc.partition_id_tensor else None
    in_names, out_names, out_avals, zero_outs = [], [], [], []
    for alloc in nc.m.functions[0].allocations:
        if not isinstance(alloc, mybir.MemoryLocationSet):
            continue
        name = alloc.memorylocations[0].name
        if alloc.kind == "ExternalInput":
            if name != partition_name:
                in_names.append(name)
        elif alloc.kind == "ExternalOutput":
            shape = tuple(alloc.tensor_shape)
            dtype = mybir.dt.np(alloc.dtype)
            out_names.append(name)
            out_avals.append(jax.core.ShapedArray(shape, dtype))
            zero_outs.append(np.zeros(shape, dtype))
    all_in_names = list(in_names) + list(out_names)
    if partition_name is not None:
        all_in_names.append(partition_name)

    def _body(*args):
        operands = list(args)
        if partition_name is not None:
            operands.append(partition_id_tensor())
        outs = _bass_exec_p.bind(
            *operands,
            out_avals=tuple(out_avals),
            in_names=tuple(all_in_names),
            out_names=tuple(out_names),
            lowering_input_output_aliases=(),
            sim_require_finite=True,
            sim_require_nnan=True,
            nc=nc,
        )
        return tuple(outs)

    devices = jax.devices()[:N_CORES]
    mesh = Mesh(np.asarray(devices), ("core",))
    n_in = len(in_names) + len(zero_outs)
    fn = jax.jit(
        shard_map(_body, mesh=mesh,
                  in_specs=(PartitionSpec("core"),) * n_in,
                  out_specs=(PartitionSpec("core"),) * len(out_names),
                  check_rep=False),
        keep_unused=True,
    )
    _CACHE["r"] = (fn, mesh, in_names, out_names, out_avals, zero_outs)
    return _CACHE["r"]


def _prep_in_maps(X, Wq1, Wk1, Wv1, Wo1, Wq2, Wk2, Wv2, Wo2, ln_g, ln_b,
                  W1, b1, W2, b2):
    h16 = lambda a: np.ascontiguousarray(np.asarray(a), dtype=np.float16)
    f32c = lambda a: np.ascontiguousarray(np.asarray(a), dtype=np.float32)
    shared = {
        "Wq1": h16(Wq1), "Wk1": h16(Wk1), "Wv1": h16(Wv1), "Wo1": h16(Wo1),
        "Wq2": h16(Wq2), "Wk2": h16(Wk2), "Wv2": h16(Wv2), "Wo2": h16(Wo2),
        "W1C": h16(np.asarray(W1).reshape(D, F // P, P).transpose(1, 0, 2)),
        "W2": h16(W2),
        "B1": f32c(np.asarray(b1).reshape(F // P, P).T),
        "B2": f32c(np.asarray(b2).reshape(KT, P).T),
        "G": f32c(np.asarray(ln_g).reshape(KT, P).T),
        "BB": f32c(np.asarray(ln_b).reshape(KT, P).T),
        "TRIU": np.triu(np.ones((P, P), np.float16)),
    }
    Xn = np.asarray(X)
    in_maps = []
    for c in range(N_CORES):
        m = dict(shared)
        m["XT"] = h16(Xn[c // 2].T)
        in_maps.append(m)
    return in_maps


def kernel(**inputs) -> np.ndarray:
    import jax

    fn, mesh, in_names, out_names, out_avals, zero_outs = _get_runner()
    in_maps = _prep_in_maps(**inputs)
    concat_in = [
        np.concatenate([in_maps[c][name] for c in range(N_CORES)], axis=0)
        for name in in_names
    ]
    concat_zeros = [
        np.zeros((N_CORES * z.shape[0], *z.shape[1:]), z.dtype)
        for z in zero_outs
    ]
    outs = fn(*concat_in, *concat_zeros)
    jax.block_until_ready(outs)
    i_out = out_names.index("OUT")
    per_core = np.asarray(outs[i_out]).reshape(N_CORES, D, SH)
    # assemble: core c -> batch c//2, query half c%2 ; output is [B, S, D]
    result = np.empty((B, S, D), np.float32)
    for c in range(N_CORES):
        b, h = c // 2, c % 2
        result[b, h * SH:(h + 1) * SH, :] = per_core[c].T
    return result


# revision 26
# speedup vs baseline: 13399.1170x; 1.0002x over previous
"""Trainium2 Bass kernel for nn_DecoderBlock (dense transformer block).

Sharding: 8 NeuronCores = 4 batch elements x 2 sequence halves.
Each core computes layer-1 (causal MHA + LN + residual) for its whole batch
element (duplicated across the pair - avoids any collective), then layer-2
full attention + FFN only for its own 512-query half, selected with a
partition-id-driven dynamic slice.  Activations are kept feature-major
[d, s] so every projection is a plain (weights-stationary) matmul; softmax
runs on transposed scores [k, q] with denominators obtained for free from an
extra ones-column in V; LayerNorm stats (over the partition axis) come from
ones-vector matmuls on the PE.

All matmul operands are f16 (fp32 accumulation in PSUM); LN / softmax scalar
math stays fp32.  Expected end-to-end relative error vs the fp32 reference:
~1e-3.
"""
import sys
sys.path.insert(0, '/opt/trn_rl_repo')
import numpy as np
from contextlib import ExitStack

import concourse.bass as bass
import concourse.mybir as mybir
from concourse.tile import TileContext
from concourse.bass2jax import _bass_exec_p, partition_id_tensor, install_neuronx_cc_hook

f32 = mybir.dt.float32
f16 = mybir.dt.float16
AF = mybir.ActivationFunctionType

D = 1024          # d_model
S = 1024          # sequence length
B = 4             # batch
H = 16            # heads
DH = 64           # head dim
F = 4096          # ffn hidden
P = 128
KT = D // P       # 8 d-tiles
SH = 512          # per-core sequence half
EPS = 1e-5
N_CORES = 8


# ---------------------------------------------------------------------------
# walrus workaround: split multi-sem-wait instructions into single-wait NOPs
# ---------------------------------------------------------------------------
def _split_multi_waits(nc, max_waits=1):
    n_split = 0
    for fn in nc.m.functions:
        for bb in fn.blocks:
            new_insts = []
            for inst in bb.instructions:
                si = inst.sync_info
                waits = list(si.on_wait) if si is not None else []
                if len(waits) > max_waits:
                    keep = waits[-max_waits:]
                    for w in waits[:-max_waits]:
                        nop = mybir.InstNoOp(
                            name=nc.get_next_instruction_name(),
                            engine=inst.engine,
                            sync_info=mybir.SyncInfo(on_wait=[w], on_update=[]),
                            bass_nofuse=True,
                        )
                        nc.register_instruction(nop)
                        new_insts.append(nop)
                    inst.sync_info = mybir.SyncInfo(
                        on_wait=keep, on_update=list(si.on_update))
                    n_split += 1
                new_insts.append(inst)
            bb.instructions.clear()
            for i in new_insts:
                bb.add_instruction(i)
    return n_split


# ---------------------------------------------------------------------------
# the bass program (identical on all 8 cores; per-core data differs)
# ---------------------------------------------------------------------------
def build_decoder(reps=0):
    # reps<0: runtime rep count from NREPS input
    nc = bass.Bass(num_devices=N_CORES)

    XT = nc.dram_tensor("XT", [D, S], f16, kind="ExternalInput")
    Wname = {}
    for w in ("Wq1", "Wk1", "Wv1", "Wo1", "Wq2", "Wk2", "Wv2", "Wo2"):
        Wname[w] = nc.dram_tensor(w, [D, D], f16, kind="ExternalInput")
    W1C = nc.dram_tensor("W1C", [F // P, D, P], f16, kind="ExternalInput")
    W2 = nc.dram_tensor("W2", [F, D], f16, kind="ExternalInput")
    B1 = nc.dram_tensor("B1", [P, F // P], f32, kind="ExternalInput")
    B2 = nc.dram_tensor("B2", [P, KT], f32, kind="ExternalInput")
    G = nc.dram_tensor("G", [P, KT], f32, kind="ExternalInput")
    BB = nc.dram_tensor("BB", [P, KT], f32, kind="ExternalInput")
    TRIU = nc.dram_tensor("TRIU", [P, P], f16, kind="ExternalInput")
    OUT = nc.dram_tensor("OUT", [D, SH], f32, kind="ExternalOutput")

    lp = nc.allow_low_precision(reason="f16 operand kernel by design")
    lp.__enter__()
    with TileContext(nc) as tc, ExitStack() as ctx:
        # ---- pools -------------------------------------------------------
        big = ctx.enter_context(tc.tile_pool(name="big", bufs=5))   # 16KB slots
        med = ctx.enter_context(tc.tile_pool(name="med", bufs=4))   # 8KB slots
        hpool = ctx.enter_context(tc.tile_pool(name="hp", bufs=2))
        vpool = ctx.enter_context(tc.tile_pool(name="vp", bufs=1))
        wpool = ctx.enter_context(tc.tile_pool(name="wp", bufs=10))
        epool = ctx.enter_context(tc.tile_pool(name="ep", bufs=4))
        spool = ctx.enter_context(tc.tile_pool(name="sp", bufs=1))
        one = ctx.enter_context(tc.tile_pool(name="one", bufs=1))
        pp = ctx.enter_context(tc.tile_pool(name="pp", bufs=4, space="PSUM"))

        # ---- constants / small inputs -----------------------------------
        triu = one.tile([P, P], f16, tag="triu")
        nc.sync.dma_start(out=triu, in_=TRIU[:, :])
        b1_sb = one.tile([P, F // P], f32, tag="b1")
        nc.sync.dma_start(out=b1_sb, in_=B1[:, :])
        b2_sb = one.tile([P, KT], f32, tag="b2")
        nc.sync.dma_start(out=b2_sb, in_=B2[:, :])
        g_sb = one.tile([P, KT], f32, tag="g")
        nc.sync.dma_start(out=g_sb, in_=G[:, :])
        bb_sb = one.tile([P, KT], f32, tag="bb")
        nc.sync.dma_start(out=bb_sb, in_=BB[:, :])
        ones_col = one.tile([P, 1], f16, tag="onescol")   # colsum lhsT
        nc.vector.memset(ones_col, 1.0)
        ones_row = one.tile([1, P], f16, tag="onesrow")   # bcast lhsT
        nc.vector.memset(ones_row, 1.0)
        eps_t = one.tile([1, 1], f32, tag="eps")
        nc.vector.memset(eps_t, EPS)
        eb1_t = one.tile([P, 1], f32, tag="eb1")
        nc.vector.memset(eb1_t, -2.0)
        eb2_t = one.tile([P, 1], f32, tag="eb2")
        nc.vector.memset(eb2_t, -8.0)

        rep_cm = tc.For_i(0, reps, 1) if reps else None
        if rep_cm is not None:
            rep_cm.__enter__()

        # ---- load X^T ----------------------------------------------------
        xt = big.tile([P, KT, S], f16, tag="big")
        nc.sync.dma_start(out=xt, in_=XT.rearrange("(k p) s -> p k s", p=P))

        # ---- helpers -----------------------------------------------------
        def load_w_slabs(W):
            slabs = []
            for k in range(KT):
                w = wpool.tile([P, D], f16, tag="w")
                nc.sync.dma_start(out=w, in_=W[k * P:(k + 1) * P, :])
                slabs.append(w)
            return slabs

        def project(dst, slabs, rhs_fn, n_chunks):
            """dst[:, m, n*512:+512] = W^T @ rhs ; rhs_fn(k, n) -> [128,512].
            Both 512-col chunks are issued back-to-back under the same
            stationary operand so the PE reuses/overlaps the weight load."""
            for m in range(KT):
                pss = [pp.tile([P, 512], f32, tag="ps", name=f"ps{n}")
                       for n in range(n_chunks)]
                for k in range(KT):
                    for n in range(n_chunks):
                        nc.tensor.matmul(
                            pss[n], slabs[k][:, m * P:(m + 1) * P], rhs_fn(k, n),
                            start=(k == 0), stop=(k == KT - 1))
                for n in range(n_chunks):
                    nc.vector.tensor_copy(
                        dst[:, m, n * 512:n * 512 + 512], pss[n])

        def project_v(dst4, wv_slabs):
            """Seq-major V with interleaved ones columns."""
            nc.vector.memset(dst4[:, :, :, DH:DH + 1], 1.0)
            for si in range(KT):
                pss = [pp.tile([P, 512], f32, tag="ps", name=f"psv{n}")
                       for n in range(2)]
                for k in range(KT):
                    for n in range(2):
                        nc.tensor.matmul(
                            pss[n], xt[:, k, si * P:(si + 1) * P],
                            wv_slabs[k][:, n * 512:(n + 1) * 512],
                            start=(k == 0), stop=(k == KT - 1))
                for n in range(2):
                    nc.vector.tensor_copy(
                        dst4[:, si, 8 * n:8 * n + 8, 0:DH],
                        pss[n].rearrange("p (h c) -> p h c", c=DH))

        def attention(qt, kt_sb, v_sb, attnT, n_q, causal, exp_bias=None):
            """qt [P,KT,n_q], kt_sb [P,KT,S], v_sb [P,KT,H*65] (seq-major,
            ones col), attnT [P,KT,n_q] f16 out.  Head pairs share a d-tile;
            their score tiles live in one 2-bank psum tile so the exp runs as
            a single [128, 1024] ACT op."""
            for qb in range(n_q // 512):
                qc = slice(qb * 512, qb * 512 + 512)
                for pr in range(H // 2):
                    m = pr
                    kts = list(range(min(KT, (qb + 1) * 4))) if causal \
                        else list(range(KT))
                    avs = [pp.tile([P, 512], f32, tag="ps", name=f"av{par}") for par in range(2)]
                    for k in kts:
                        c0 = max(0, k * P - qb * 512) if causal else 0
                        sc = pp.tile([P, 1024], f32, tag="ps2", bufs=2)
                        for par in range(2):
                            off = 64 * par
                            nc.tensor.matmul(
                                sc[:, 512 * par + c0:512 * par + 512],
                                kt_sb[off:off + 64, m, k * P:(k + 1) * P],
                                qt[off:off + 64, m, qb * 512 + c0:qb * 512 + 512],
                                start=True, stop=True)
                        et = epool.tile([P, 1024], f16, tag="et")
                        if c0 == 0:
                            nc.scalar.activation(
                                out=et, in_=sc, func=AF.Exp, scale=0.125,
                                bias=exp_bias)
                        else:
                            for par in range(2):
                                nc.scalar.activation(
                                    out=et[:, 512 * par + c0:512 * par + 512],
                                    in_=sc[:, 512 * par + c0:512 * par + 512],
                                    func=AF.Exp, scale=0.125, bias=exp_bias)
                        if causal and k >= qb * 4:
                            for par in range(2):
                                nc.vector.tensor_mul(
                                    et[:, 512 * par + c0:512 * par + c0 + P],
                                    et[:, 512 * par + c0:512 * par + c0 + P],
                                    triu)
                        for par in range(2):
                            h16 = 2 * pr + par
                            nc.tensor.matmul(
                                avs[par][0:DH + 1, c0:512],
                                v_sb[:, k, h16 * 65:h16 * 65 + 65],
                                et[:, 512 * par + c0:512 * par + 512],
                                start=(k == kts[0]), stop=(k == kts[-1]))
                    for par in range(2):
                        rd = spool.tile([1, 512], f16, tag="rd", bufs=4)
                        nc.vector.reciprocal(rd, avs[par][DH:DH + 1, :])
                        den = pp.tile([P, 512], f32, tag="ps")
                        nc.tensor.matmul(den[0:64, :], ones_row[0:1, 0:64], rd,
                                         start=True, stop=True)
                        den_sb = spool.tile([64, 512], f32, tag="densb", bufs=2)
                        nc.vector.tensor_copy(den_sb, den[0:64, :])
                        if par == 0:
                            nc.vector.tensor_mul(
                                attnT[0:64, m, qc], avs[par][0:64, :], den_sb)
                        else:
                            avn = spool.tile([64, 512], f16, tag="avn", bufs=3)
                            nc.vector.tensor_mul(avn, avs[par][0:64, :], den_sb)
                            nc.sync.dma_start(out=attnT[64:128, m, qc], in_=avn)

        def layernorm_resid(src, resid, dst, n_cols, final_f32=None):
            """dst = resid + LN(src); LN over the partition (d) axis.
            src/resid/dst are [P, KT, n_cols] f16 tiles (dst may be resid)."""
            n_chunks = n_cols // 512
            stats = []
            for n in range(n_chunks):
                st_a = pp.tile([1, 512], f32, tag="ps")
                st_b = pp.tile([1, 512], f32, tag="ps")
                for m in range(KT):
                    cc = slice(n * 512, n * 512 + 512)
                    sq = spool.tile([P, 512], f16, tag="sq", bufs=2)
                    nc.vector.tensor_mul(sq, src[:, m, cc], src[:, m, cc])
                    nc.tensor.matmul(st_a, ones_col, src[:, m, cc],
                                     start=(m == 0), stop=(m == KT - 1))
                    nc.tensor.matmul(st_b, ones_col, sq,
                                     start=(m == 0), stop=(m == KT - 1))
                stats.append((st_a, st_b))
            for n in range(n_chunks):
                st_a, st_b = stats[n]
                cc = slice(n * 512, n * 512 + 512)
                mu = spool.tile([1, 512], f32, tag="r32", bufs=4)
                nc.scalar.mul(mu, st_a, 1.0 / D)
                msq = spool.tile([1, 512], f32, tag="r32", bufs=4)
                nc.scalar.mul(msq, st_b, 1.0 / D)
                mu2 = spool.tile([1, 512], f32, tag="r32", bufs=4)
                nc.vector.tensor_mul(mu2, mu, mu)
                var = spool.tile([1, 512], f32, tag="r32", bufs=4)
                nc.vector.tensor_sub(var, msq, mu2)
                sd = spool.tile([1, 512], f32, tag="r32", bufs=4)
                nc.scalar.activation(out=sd, in_=var, func=AF.Sqrt,
                                     bias=eps_t, scale=1.0)
                rs32 = spool.tile([1, 512], f32, tag="r32", bufs=4)
                nc.vector.reciprocal(rs32, sd)
                rs16 = spool.tile([1, 512], f16, tag="rd", bufs=4)
                nc.vector.tensor_copy(rs16, rs32)
                qrow = spool.tile([1, 512], f16, tag="rd", bufs=4)
                nc.vector.tensor_mul(qrow, mu, rs32)
                pb_ps = pp.tile([P, 512], f32, tag="ps")
                nc.tensor.matmul(pb_ps, ones_row[0:1, 0:P], rs16,
                                 start=True, stop=True)
                qb_ps = pp.tile([P, 512], f32, tag="ps")
                nc.tensor.matmul(qb_ps, ones_row[0:1, 0:P], qrow,
                                 start=True, stop=True)
                pb16 = spool.tile([P, 512], f16, tag="pb16", bufs=2)
                nc.vector.tensor_copy(pb16, pb_ps)
                qb32 = spool.tile([P, 512], f32, tag="qb32", bufs=2)
                nc.vector.tensor_copy(qb32, qb_ps)
                for m in range(KT):
                    t1 = spool.tile([P, 512], f32, tag="t1", bufs=2)
                    nc.vector.tensor_mul(t1, src[:, m, cc], pb16)
                    nc.vector.tensor_sub(t1, t1, qb32)
                    t3 = spool.tile([P, 512], f16, tag="t3", bufs=3)
                    nc.scalar.activation(out=t3, in_=t1, func=AF.Identity,
                                         bias=bb_sb[:, m:m + 1],
                                         scale=g_sb[:, m:m + 1])
                    if final_f32 is not None:
                        nc.vector.tensor_add(final_f32[:, m, :],
                                             resid[:, m, cc], t3)
                    else:
                        nc.vector.tensor_add(dst[:, m, cc],
                                             resid[:, m, cc], t3)

        # ================= layer 1: causal MHA (full batch) ==============
        qt = big.tile([P, KT, S], f16, tag="big")
        project(qt, load_w_slabs(Wname["Wq1"]),
                lambda k, n: xt[:, k, n * 512:(n + 1) * 512], 2)
        kt_sb = big.tile([P, KT, S], f16, tag="big")
        project(kt_sb, load_w_slabs(Wname["Wk1"]),
                lambda k, n: xt[:, k, n * 512:(n + 1) * 512], 2)
        v_sb = vpool.tile([P, KT, H * (DH + 1)], f16, tag="v")
        project_v(v_sb.rearrange("p k (h c) -> p k h c", c=DH + 1),
                  load_w_slabs(Wname["Wv1"]))

        attnT = big.tile([P, KT, S], f16, tag="big")
        attention(qt, kt_sb, v_sb, attnT, S, causal=True, exp_bias=eb1_t)

        masked = big.tile([P, KT, S], f16, tag="big")
        project(masked, load_w_slabs(Wname["Wo1"]),
                lambda k, n: attnT[:, k, n * 512:(n + 1) * 512], 2)
        layernorm_resid(masked, xt, xt, S)      # xt <- norm_masked

        # ================= layer 2: full MHA (own q-half) ================
        # q-half of this core (0 or 512): dynamic-slice norm_masked on DVE
        qlo_v = (nc.vector.partition_id() % 2) * SH
        nmq = med.tile([P, KT, SH], f16, tag="med")
        xt_flat = xt.rearrange("p k s -> p (k s)")
        for k in range(KT):
            nc.vector.tensor_copy(nmq[:, k, :],
                                  xt_flat[:, bass.ds(qlo_v + k * S, 512)])
        q2t = med.tile([P, KT, SH], f16, tag="med")
        project(q2t, load_w_slabs(Wname["Wq2"]),
                lambda k, n: nmq[:, k, :], 1)
        k2t = big.tile([P, KT, S], f16, tag="big")
        project(k2t, load_w_slabs(Wname["Wk2"]),
                lambda k, n: xt[:, k, n * 512:(n + 1) * 512], 2)
        v2_sb = vpool.tile([P, KT, H * (DH + 1)], f16, tag="v")
        project_v(v2_sb.rearrange("p k (h c) -> p k h c", c=DH + 1),
                  load_w_slabs(Wname["Wv2"]))

        attn2T = med.tile([P, KT, SH], f16, tag="med")
        attention(q2t, k2t, v2_sb, attn2T, SH, causal=False, exp_bias=eb2_t)

        attn2 = med.tile([P, KT, SH], f16, tag="med")
        project(attn2, load_w_slabs(Wname["Wo2"]),
                lambda k, n: attn2T[:, k, 0:512], 1)
        attn2n = med.tile([P, KT, SH], f16, tag="med")
        layernorm_resid(attn2, attn2, attn2n, SH)

        # ================= FFN (own q-half) ==============================
        ff_acc = big.tile([P, KT, SH], f32, tag="big")
        for hc in range(4):                     # 4 hidden chunks of 1024
            h_sb = hpool.tile([P, 8, 512], f16, tag="h")
            for hm in range(8):
                m32 = hc * 8 + hm
                w1t = wpool.tile([P, KT, P], f16, tag="w")
                nc.sync.dma_start(
                    out=w1t, in_=W1C[m32].rearrange("(k p) c -> p k c", p=P))
                ps = pp.tile([P, 512], f32, tag="ps")
                for k in range(KT):
                    nc.tensor.matmul(ps, w1t[:, k, :], attn2n[:, k, 0:512],
                                     start=(k == 0), stop=(k == KT - 1))
                nc.scalar.activation(out=h_sb[:, hm, :], in_=ps, func=AF.Relu,
                                     bias=b1_sb[:, m32:m32 + 1], scale=1.0)
            w2sl = []
            for kk in range(8):
                h32 = hc * 8 + kk
                w2s = wpool.tile([P, D], f16, tag="w")
                nc.sync.dma_start(out=w2s, in_=W2[h32 * P:(h32 + 1) * P, :])
                w2sl.append(w2s)
            for m in range(KT):
                ps = pp.tile([P, 512], f32, tag="ps")
                for kk in range(8):
                    nc.tensor.matmul(ps, w2sl[kk][:, m * P:(m + 1) * P],
                                     h_sb[:, kk, :],
                                     start=(kk == 0), stop=(kk == 7))
                if hc == 0:
                    nc.vector.tensor_copy(ff_acc[:, m, 0:512], ps)
                else:
                    nc.vector.tensor_add(ff_acc[:, m, 0:512],
                                         ff_acc[:, m, 0:512], ps)
        ff = med.tile([P, KT, SH], f16, tag="med")
        for m in range(KT):
            nc.scalar.activation(out=ff[:, m, :], in_=ff_acc[:, m, 0:512],
                                 func=AF.Identity, bias=b2_sb[:, m:m + 1],
                                 scale=1.0)
        final = big.tile([P, KT, SH], f32, tag="big")
        layernorm_resid(ff, attn2n, None, SH, final_f32=final)

        for m in range(KT):
            nc.sync.dma_start(out=OUT[m * P:(m + 1) * P, :], in_=final[:, m, :])

        if rep_cm is not None:
            rep_cm.__exit__(None, None, None)

    lp.__exit__(None, None, None)
    _split_multi_waits(nc)
    return nc


# ---------------------------------------------------------------------------
# host wrapper: compile once, shard inputs, run on 8 cores, gather
# ---------------------------------------------------------------------------
_CACHE = {}


def _get_runner():
    if "r" in _CACHE:
        return _CACHE["r"]
    import jax
    from jax.sharding import Mesh, PartitionSpec
    from jax.experimental.shard_map import shard_map

    install_neuronx_cc_hook()
    nc = build_decoder()
    partition_name = nc.partition_id_tensor.name if nc.partition_id_tensor else None
    in_names, out_names, out_avals, zero_outs = [], [], [], []
    for alloc in nc.m.functions[0].allocations:
        if not isinstance(alloc, mybir.MemoryLocationSet):
            continue
        name = alloc.memorylocations[0].name
        if alloc.kind == "ExternalInput":
            if name != partition_name:
                in_names.append(name)
        elif alloc.kind == "ExternalOutput":
            shape = tuple(alloc.tensor_shape)
            dtype = mybir.dt.np(alloc.dtype)
            out_names.append(name)
            out_avals.append(jax.core.ShapedArray(shape, dtype))
            zero_outs.append(np.zeros(shape, dtype))
    all_in_names = list(in_names) + list(out_names)
    if partition_name is not None:
        all_in_names.append(partition_name)

    def _body(*args):
        operands = list(args)
        if partition_name is not None:
            operands.append(partition_id_tensor())
        outs = _bass_exec_p.bind(
            *operands,
            out_avals=tuple(out_avals),
            in_names=tuple(all_in_names),
            out_names=tuple(out_names),
            lowering_input_output_aliases=(),
            sim_require_finite=True,
            sim_require_nnan=True,
            nc=nc,
        )
        return tuple(outs)

    devices = jax.devices()[:N_CORES]
    mesh = Mesh(np.asarray(devices), ("core",))
    n_in = len(in_names) + len(zero_outs)
    fn = jax.jit(
        shard_map(_body, mesh=mesh,
                  in_specs=(PartitionSpec("core"),) * n_in,
                  out_specs=(PartitionSpec("core"),) * len(out_names),
                  check_rep=False),
        keep_unused=True,
    )
    _CACHE["r"] = (fn, mesh, in_names, out_names, out_avals, zero_outs)
    return _CACHE["r"]


def _prep_in_maps(X, Wq1, Wk1, Wv1, Wo1, Wq2, Wk2, Wv2, Wo2, ln_g, ln_b,
                  W1, b1, W2, b2):
    h16 = lambda a: np.ascontiguousarray(np.asarray(a), dtype=np.float16)
    f32c = lambda a: np.ascontiguousarray(np.asarray(a), dtype=np.float32)
    shared = {
        "Wq1": h16(Wq1), "Wk1": h16(Wk1), "Wv1": h16(Wv1), "Wo1": h16(Wo1),
        "Wq2": h16(Wq2), "Wk2": h16(Wk2), "Wv2": h16(Wv2), "Wo2": h16(Wo2),
        "W1C": h16(np.asarray(W1).reshape(D, F // P, P).transpose(1, 0, 2)),
        "W2": h16(W2),
        "B1": f32c(np.asarray(b1).reshape(F // P, P).T),
        "B2": f32c(np.asarray(b2).reshape(KT, P).T),
        "G": f32c(np.asarray(ln_g).reshape(KT, P).T),
        "BB": f32c(np.asarray(ln_b).reshape(KT, P).T),
        "TRIU": np.triu(np.ones((P, P), np.float16)),
    }
    Xn = np.asarray(X)
    in_maps = []
    for c in range(N_CORES):
        m = dict(shared)
        m["XT"] = h16(Xn[c // 2].T)
        in_maps.append(m)
    return in_maps


def kernel(**inputs) -> np.ndarray:
    import jax

    fn, mesh, in_names, out_names, out_avals, zero_outs = _get_runner()
    in_maps = _prep_in_maps(**inputs)
    concat_in = [
        np.concatenate([in_maps[c][name] for c in range(N_CORES)], axis=0)
        for name in in_names
    ]
    concat_zeros = [
        np.zeros((N_CORES * z.shape[0], *z.shape[1:]), z.dtype)
        for z in zero_outs
    ]
    outs = fn(*concat_in, *concat_zeros)
    jax.block_until_ready(outs)
    i_out = out_names.index("OUT")
    per_core = np.asarray(outs[i_out]).reshape(N_CORES, D, SH)
    # assemble: core c -> batch c//2, query half c%2 ; output is [B, S, D]
    result = np.empty((B, S, D), np.float32)
    for c in range(N_CORES):
        b, h = c // 2, c % 2
        result[b, h * SH:(h + 1) * SH, :] = per_core[c].T
    return result


# revision 27
# speedup vs baseline: 13561.0818x; 1.0121x over previous
"""Trainium2 Bass kernel for nn_DecoderBlock (dense transformer block).

Sharding: 8 NeuronCores = 4 batch elements x 2 sequence halves.
Each core computes layer-1 (causal MHA + LN + residual) for its whole batch
element (duplicated across the pair - avoids any collective), then layer-2
full attention + FFN only for its own 512-query half, selected with a
partition-id-driven dynamic slice.  Activations are kept feature-major
[d, s] so every projection is a plain (weights-stationary) matmul; softmax
runs on transposed scores [k, q] with denominators obtained for free from an
extra ones-column in V; LayerNorm stats (over the partition axis) come from
ones-vector matmuls on the PE.

All matmul operands are f16 (fp32 accumulation in PSUM); LN / softmax scalar
math stays fp32.  Expected end-to-end relative error vs the fp32 reference:
~1e-3.
"""
import sys
sys.path.insert(0, '/opt/trn_rl_repo')
import numpy as np
from contextlib import ExitStack

import concourse.bass as bass
import concourse.mybir as mybir
from concourse.tile import TileContext
from concourse.bass2jax import _bass_exec_p, partition_id_tensor, install_neuronx_cc_hook

f32 = mybir.dt.float32
f16 = mybir.dt.float16
AF = mybir.ActivationFunctionType

D = 1024          # d_model
S = 1024          # sequence length
B = 4             # batch
H = 16            # heads
DH = 64           # head dim
F = 4096          # ffn hidden
P = 128
KT = D // P       # 8 d-tiles
SH = 512          # per-core sequence half
EPS = 1e-5
N_CORES = 8


# ---------------------------------------------------------------------------
# walrus workaround: split multi-sem-wait instructions into single-wait NOPs
# ---------------------------------------------------------------------------
def _split_multi_waits(nc, max_waits=1):
    n_split = 0
    for fn in nc.m.functions:
        for bb in fn.blocks:
            new_insts = []
            for inst in bb.instructions:
                si = inst.sync_info
                waits = list(si.on_wait) if si is not None else []
                if len(waits) > max_waits:
                    keep = waits[-max_waits:]
                    for w in waits[:-max_waits]:
                        nop = mybir.InstNoOp(
                            name=nc.get_next_instruction_name(),
                            engine=inst.engine,
                            sync_info=mybir.SyncInfo(on_wait=[w], on_update=[]),
                            bass_nofuse=True,
                        )
                        nc.register_instruction(nop)
                        new_insts.append(nop)
                    inst.sync_info = mybir.SyncInfo(
                        on_wait=keep, on_update=list(si.on_update))
                    n_split += 1
                new_insts.append(inst)
            bb.instructions.clear()
            for i in new_insts:
                bb.add_instruction(i)
    return n_split


# ---------------------------------------------------------------------------
# the bass program (identical on all 8 cores; per-core data differs)
# ---------------------------------------------------------------------------
def build_decoder(reps=0):
    # reps<0: runtime rep count from NREPS input
    nc = bass.Bass(num_devices=N_CORES)

    XT = nc.dram_tensor("XT", [D, S], f16, kind="ExternalInput")
    Wname = {}
    for w in ("Wq1", "Wk1", "Wv1", "Wo1", "Wq2", "Wk2", "Wv2", "Wo2"):
        Wname[w] = nc.dram_tensor(w, [D, D], f16, kind="ExternalInput")
    W1C = nc.dram_tensor("W1C", [F // P, D, P], f16, kind="ExternalInput")
    W2 = nc.dram_tensor("W2", [F, D], f16, kind="ExternalInput")
    B1 = nc.dram_tensor("B1", [P, F // P], f32, kind="ExternalInput")
    B2 = nc.dram_tensor("B2", [P, KT], f32, kind="ExternalInput")
    G = nc.dram_tensor("G", [P, KT], f32, kind="ExternalInput")
    BB = nc.dram_tensor("BB", [P, KT], f32, kind="ExternalInput")
    TRIU = nc.dram_tensor("TRIU", [P, P], f16, kind="ExternalInput")
    OUT = nc.dram_tensor("OUT", [D, SH], f32, kind="ExternalOutput")

    lp = nc.allow_low_precision(reason="f16 operand kernel by design")
    lp.__enter__()
    with TileContext(nc) as tc, ExitStack() as ctx:
        # ---- pools -------------------------------------------------------
        big = ctx.enter_context(tc.tile_pool(name="big", bufs=5))   # 16KB slots
        med = ctx.enter_context(tc.tile_pool(name="med", bufs=4))   # 8KB slots
        hpool = ctx.enter_context(tc.tile_pool(name="hp", bufs=2))
        vpool = ctx.enter_context(tc.tile_pool(name="vp", bufs=1))
        wpool = ctx.enter_context(tc.tile_pool(name="wp", bufs=10))
        epool = ctx.enter_context(tc.tile_pool(name="ep", bufs=4))
        spool = ctx.enter_context(tc.tile_pool(name="sp", bufs=1))
        one = ctx.enter_context(tc.tile_pool(name="one", bufs=1))
        pp = ctx.enter_context(tc.tile_pool(name="pp", bufs=4, space="PSUM"))

        # ---- constants / small inputs -----------------------------------
        triu = one.tile([P, P], f16, tag="triu")
        nc.sync.dma_start(out=triu, in_=TRIU[:, :])
        b1_sb = one.tile([P, F // P], f32, tag="b1")
        nc.sync.dma_start(out=b1_sb, in_=B1[:, :])
        b2_sb = one.tile([P, KT], f32, tag="b2")
        nc.sync.dma_start(out=b2_sb, in_=B2[:, :])
        g_sb = one.tile([P, KT], f32, tag="g")
        nc.sync.dma_start(out=g_sb, in_=G[:, :])
        bb_sb = one.tile([P, KT], f32, tag="bb")
        nc.sync.dma_start(out=bb_sb, in_=BB[:, :])
        ones_col = one.tile([P, 1], f16, tag="onescol")   # colsum lhsT
        nc.vector.memset(ones_col, 1.0)
        ones_row = one.tile([1, P], f16, tag="onesrow")   # bcast lhsT
        nc.vector.memset(ones_row, 1.0)
        eps_t = one.tile([1, 1], f32, tag="eps")
        nc.vector.memset(eps_t, EPS)
        eb1_t = one.tile([P, 1], f32, tag="eb1")
        nc.vector.memset(eb1_t, -2.0)
        eb2_t = one.tile([P, 1], f32, tag="eb2")
        nc.vector.memset(eb2_t, -8.0)

        rep_cm = tc.For_i(0, reps, 1) if reps else None
        if rep_cm is not None:
            rep_cm.__enter__()

        # ---- load X^T ----------------------------------------------------
        xt = big.tile([P, KT, S], f16, tag="big")
        nc.sync.dma_start(out=xt, in_=XT.rearrange("(k p) s -> p k s", p=P))

        # ---- helpers -----------------------------------------------------
        def load_w_slabs(W):
            slabs = []
            for k in range(KT):
                w = wpool.tile([P, D], f16, tag="w")
                nc.sync.dma_start(out=w, in_=W[k * P:(k + 1) * P, :])
                slabs.append(w)
            return slabs

        def project(dst, slabs, rhs_fn, n_chunks):
            """dst[:, m, n*512:+512] = W^T @ rhs ; rhs_fn(k, n) -> [128,512].
            Both 512-col chunks are issued back-to-back under the same
            stationary operand so the PE reuses/overlaps the weight load."""
            for m in range(KT):
                pss = [pp.tile([P, 512], f32, tag="ps", name=f"ps{n}")
                       for n in range(n_chunks)]
                for k in range(KT):
                    for n in range(n_chunks):
                        nc.tensor.matmul(
                            pss[n], slabs[k][:, m * P:(m + 1) * P], rhs_fn(k, n),
                            start=(k == 0), stop=(k == KT - 1))
                for n in range(n_chunks):
                    nc.vector.tensor_copy(
                        dst[:, m, n * 512:n * 512 + 512], pss[n])

        def project_v(dst4, wv_slabs):
            """Seq-major V with interleaved ones columns."""
            nc.vector.memset(dst4[:, :, :, DH:DH + 1], 1.0)
            for si in range(KT):
                pss = [pp.tile([P, 512], f32, tag="ps", name=f"psv{n}")
                       for n in range(2)]
                for k in range(KT):
                    for n in range(2):
                        nc.tensor.matmul(
                            pss[n], xt[:, k, si * P:(si + 1) * P],
                            wv_slabs[k][:, n * 512:(n + 1) * 512],
                            start=(k == 0), stop=(k == KT - 1))
                for n in range(2):
                    nc.vector.tensor_copy(
                        dst4[:, si, 8 * n:8 * n + 8, 0:DH],
                        pss[n].rearrange("p (h c) -> p h c", c=DH))

        def attention(qt, kt_sb, v_sb, attnT, n_q, causal, exp_bias=None):
            """qt [P,KT,n_q], kt_sb [P,KT,S], v_sb [P,KT,H*65] (seq-major,
            ones col), attnT [P,KT,n_q] f16 out.  Head pairs share a d-tile;
            their score tiles live in one 2-bank psum tile so the exp runs as
            a single [128, 1024] ACT op."""
            for qb in range(n_q // 512):
                qc = slice(qb * 512, qb * 512 + 512)
                for pr in range(H // 2):
                    m = pr
                    kts = list(range(min(KT, (qb + 1) * 4))) if causal \
                        else list(range(KT))
                    avs = [pp.tile([P, 512], f32, tag="ps", name=f"av{par}") for par in range(2)]
                    for k in kts:
                        c0 = max(0, k * P - qb * 512) if causal else 0
                        sc = pp.tile([P, 1024], f32, tag="ps2", bufs=2)
                        for par in range(2):
                            off = 64 * par
                            nc.tensor.matmul(
                                sc[:, 512 * par + c0:512 * par + 512],
                                kt_sb[off:off + 64, m, k * P:(k + 1) * P],
                                qt[off:off + 64, m, qb * 512 + c0:qb * 512 + 512],
                                start=True, stop=True)
                        et = epool.tile([P, 1024], f16, tag="et")
                        if c0 == 0:
                            nc.scalar.activation(
                                out=et, in_=sc, func=AF.Exp, scale=0.125,
                                bias=exp_bias)
                        else:
                            for par in range(2):
                                nc.scalar.activation(
                                    out=et[:, 512 * par + c0:512 * par + 512],
                                    in_=sc[:, 512 * par + c0:512 * par + 512],
                                    func=AF.Exp, scale=0.125, bias=exp_bias)
                        if causal and k >= qb * 4:
                            for par in range(2):
                                nc.vector.tensor_mul(
                                    et[:, 512 * par + c0:512 * par + c0 + P],
                                    et[:, 512 * par + c0:512 * par + c0 + P],
                                    triu)
                        for par in range(2):
                            h16 = 2 * pr + par
                            nc.tensor.matmul(
                                avs[par][0:DH + 1, c0:512],
                                v_sb[:, k, h16 * 65:h16 * 65 + 65],
                                et[:, 512 * par + c0:512 * par + 512],
                                start=(k == kts[0]), stop=(k == kts[-1]))
                    for par in range(2):
                        rd = spool.tile([1, 512], f16, tag="rd", bufs=4)
                        nc.vector.reciprocal(rd, avs[par][DH:DH + 1, :])
                        den = pp.tile([P, 512], f32, tag="ps")
                        nc.tensor.matmul(den[0:64, :], ones_row[0:1, 0:64], rd,
                                         start=True, stop=True)
                        den_sb = spool.tile([64, 512], f32, tag="densb", bufs=2)
                        nc.vector.tensor_copy(den_sb, den[0:64, :])
                        off = 64 * par
                        nc.vector.tensor_mul(
                            attnT[off:off + 64, m, qc], avs[par][0:64, :], den_sb)

        def layernorm_resid(src, resid, dst, n_cols, final_f32=None):
            """dst = resid + LN(src); LN over the partition (d) axis.
            src/resid/dst are [P, KT, n_cols] f16 tiles (dst may be resid)."""
            n_chunks = n_cols // 512
            stats = []
            for n in range(n_chunks):
                st_a = pp.tile([1, 512], f32, tag="ps")
                st_b = pp.tile([1, 512], f32, tag="ps")
                for m in range(KT):
                    cc = slice(n * 512, n * 512 + 512)
                    sq = spool.tile([P, 512], f16, tag="sq", bufs=2)
                    nc.vector.tensor_mul(sq, src[:, m, cc], src[:, m, cc])
                    nc.tensor.matmul(st_a, ones_col, src[:, m, cc],
                                     start=(m == 0), stop=(m == KT - 1))
                    nc.tensor.matmul(st_b, ones_col, sq,
                                     start=(m == 0), stop=(m == KT - 1))
                stats.append((st_a, st_b))
            for n in range(n_chunks):
                st_a, st_b = stats[n]
                cc = slice(n * 512, n * 512 + 512)
                mu = spool.tile([1, 512], f32, tag="r32", bufs=4)
                nc.scalar.mul(mu, st_a, 1.0 / D)
                msq = spool.tile([1, 512], f32, tag="r32", bufs=4)
                nc.scalar.mul(msq, st_b, 1.0 / D)
                mu2 = spool.tile([1, 512], f32, tag="r32", bufs=4)
                nc.vector.tensor_mul(mu2, mu, mu)
                var = spool.tile([1, 512], f32, tag="r32", bufs=4)
                nc.vector.tensor_sub(var, msq, mu2)
                sd = spool.tile([1, 512], f32, tag="r32", bufs=4)
                nc.scalar.activation(out=sd, in_=var, func=AF.Sqrt,
                                     bias=eps_t, scale=1.0)
                rs32 = spool.tile([1, 512], f32, tag="r32", bufs=4)
                nc.vector.reciprocal(rs32, sd)
                rs16 = spool.tile([1, 512], f16, tag="rd", bufs=4)
                nc.vector.tensor_copy(rs16, rs32)
                qrow = spool.tile([1, 512], f16, tag="rd", bufs=4)
                nc.vector.tensor_mul(qrow, mu, rs32)
                pb_ps = pp.tile([P, 512], f32, tag="ps")
                nc.tensor.matmul(pb_ps, ones_row[0:1, 0:P], rs16,
                                 start=True, stop=True)
                qb_ps = pp.tile([P, 512], f32, tag="ps")
                nc.tensor.matmul(qb_ps, ones_row[0:1, 0:P], qrow,
                                 start=True, stop=True)
                pb16 = spool.tile([P, 512], f16, tag="pb16", bufs=2)
                nc.vector.tensor_copy(pb16, pb_ps)
                qb32 = spool.tile([P, 512], f32, tag="qb32", bufs=2)
                nc.vector.tensor_copy(qb32, qb_ps)
                for m in range(KT):
                    t1 = spool.tile([P, 512], f32, tag="t1", bufs=2)
                    nc.vector.tensor_mul(t1, src[:, m, cc], pb16)
                    nc.vector.tensor_sub(t1, t1, qb32)
                    t3 = spool.tile([P, 512], f16, tag="t3", bufs=3)
                    nc.scalar.activation(out=t3, in_=t1, func=AF.Identity,
                                         bias=bb_sb[:, m:m + 1],
                                         scale=g_sb[:, m:m + 1])
                    if final_f32 is not None:
                        nc.vector.tensor_add(final_f32[:, m, :],
                                             resid[:, m, cc], t3)
                    else:
                        nc.vector.tensor_add(dst[:, m, cc],
                                             resid[:, m, cc], t3)

        # ================= layer 1: causal MHA (full batch) ==============
        qt = big.tile([P, KT, S], f16, tag="big")
        project(qt, load_w_slabs(Wname["Wq1"]),
                lambda k, n: xt[:, k, n * 512:(n + 1) * 512], 2)
        kt_sb = big.tile([P, KT, S], f16, tag="big")
        project(kt_sb, load_w_slabs(Wname["Wk1"]),
                lambda k, n: xt[:, k, n * 512:(n + 1) * 512], 2)
        v_sb = vpool.tile([P, KT, H * (DH + 1)], f16, tag="v")
        project_v(v_sb.rearrange("p k (h c) -> p k h c", c=DH + 1),
                  load_w_slabs(Wname["Wv1"]))

        attnT = big.tile([P, KT, S], f16, tag="big")
        attention(qt, kt_sb, v_sb, attnT, S, causal=True, exp_bias=eb1_t)

        masked = big.tile([P, KT, S], f16, tag="big")
        project(masked, load_w_slabs(Wname["Wo1"]),
                lambda k, n: attnT[:, k, n * 512:(n + 1) * 512], 2)
        layernorm_resid(masked, xt, xt, S)      # xt <- norm_masked

        # ================= layer 2: full MHA (own q-half) ================
        # q-half of this core (0 or 512): dynamic-slice norm_masked on DVE
        qlo_v = (nc.vector.partition_id() % 2) * SH
        nmq = med.tile([P, KT, SH], f16, tag="med")
        xt_flat = xt.rearrange("p k s -> p (k s)")
        for k in range(KT):
            nc.vector.tensor_copy(nmq[:, k, :],
                                  xt_flat[:, bass.ds(qlo_v + k * S, 512)])
        q2t = med.tile([P, KT, SH], f16, tag="med")
        project(q2t, load_w_slabs(Wname["Wq2"]),
                lambda k, n: nmq[:, k, :], 1)
        k2t = big.tile([P, KT, S], f16, tag="big")
        project(k2t, load_w_slabs(Wname["Wk2"]),
                lambda k, n: xt[:, k, n * 512:(n + 1) * 512], 2)
        v2_sb = vpool.tile([P, KT, H * (DH + 1)], f16, tag="v")
        project_v(v2_sb.rearrange("p k (h c) -> p k h c", c=DH + 1),
                  load_w_slabs(Wname["Wv2"]))

        attn2T = med.tile([P, KT, SH], f16, tag="med")
        attention(q2t, k2t, v2_sb, attn2T, SH, causal=False, exp_bias=eb2_t)

        attn2 = med.tile([P, KT, SH], f16, tag="med")
        project(attn2, load_w_slabs(Wname["Wo2"]),
                lambda k, n: attn2T[:, k, 0:512], 1)
        attn2n = med.tile([P, KT, SH], f16, tag="med")
        layernorm_resid(attn2, attn2, attn2n, SH)

        # ================= FFN (own q-half) ==============================
        ff_acc = big.tile([P, KT, SH], f32, tag="big")
        for hc in range(4):                     # 4 hidden chunks of 1024
            h_sb = hpool.tile([P, 8, 512], f16, tag="h")
            for hm in range(8):
                m32 = hc * 8 + hm
                w1t = wpool.tile([P, KT, P], f16, tag="w")
                nc.sync.dma_start(
                    out=w1t, in_=W1C[m32].rearrange("(k p) c -> p k c", p=P))
                ps = pp.tile([P, 512], f32, tag="ps")
                for k in range(KT):
                    nc.tensor.matmul(ps, w1t[:, k, :], attn2n[:, k, 0:512],
                                     start=(k == 0), stop=(k == KT - 1))
                nc.scalar.activation(out=h_sb[:, hm, :], in_=ps, func=AF.Relu,
                                     bias=b1_sb[:, m32:m32 + 1], scale=1.0)
            w2sl = []
            for kk in range(8):
                h32 = hc * 8 + kk
                w2s = wpool.tile([P, D], f16, tag="w")
                nc.sync.dma_start(out=w2s, in_=W2[h32 * P:(h32 + 1) * P, :])
                w2sl.append(w2s)
            for m in range(KT):
                ps = pp.tile([P, 512], f32, tag="ps")
                for kk in range(8):
                    nc.tensor.matmul(ps, w2sl[kk][:, m * P:(m + 1) * P],
                                     h_sb[:, kk, :],
                                     start=(kk == 0), stop=(kk == 7))
                if hc == 0:
                    nc.vector.tensor_copy(ff_acc[:, m, 0:512], ps)
                else:
                    nc.vector.tensor_add(ff_acc[:, m, 0:512],
                                         ff_acc[:, m, 0:512], ps)
        ff = med.tile([P, KT, SH], f16, tag="med")
        for m in range(KT):
            nc.scalar.activation(out=ff[:, m, :], in_=ff_acc[:, m, 0:512],
                                 func=AF.Identity, bias=b2_sb[:, m:m + 1],
                                 scale=1.0)
        final = big.tile([P, KT, SH], f32, tag="big")
        layernorm_resid(ff, attn2n, None, SH, final_f32=final)

        for m in range(KT):
            nc.sync.dma_start(out=OUT[m * P:(m + 1) * P, :], in_=final[:, m, :])

        if rep_cm is not None:
            rep_cm.__exit__(None, None, None)

    lp.__exit__(None, None, None)
    _split_multi_waits(nc)
    return nc


# ---------------------------------------------------------------------------
# host wrapper: compile once, shard inputs, run on 8 cores, gather
# ---------------------------------------------------------------------------
_CACHE = {}


def _get_runner():
    if "r" in _CACHE:
        return _CACHE["r"]
    import jax
    from jax.sharding import Mesh, PartitionSpec
    from jax.experimental.shard_map import shard_map

    install_neuronx_cc_hook()
    nc = build_decoder()
    partition_name = nc.partition_id_tensor.name if nc.partition_id_tensor else None
    in_names, out_names, out_avals, zero_outs = [], [], [], []
    for alloc in nc.m.functions[0].allocations:
        if not isinstance(alloc, mybir.MemoryLocationSet):
            continue
        name = alloc.memorylocations[0].name
        if alloc.kind == "ExternalInput":
            if name != partition_name:
                in_names.append(name)
        elif alloc.kind == "ExternalOutput":
            shape = tuple(alloc.tensor_shape)
            dtype = mybir.dt.np(alloc.dtype)
            out_names.append(name)
            out_avals.append(jax.core.ShapedArray(shape, dtype))
            zero_outs.append(np.zeros(shape, dtype))
    all_in_names = list(in_names) + list(out_names)
    if partition_name is not None:
        all_in_names.append(partition_name)

    def _body(*args):
        operands = list(args)
        if partition_name is not None:
            operands.append(partition_id_tensor())
        outs = _bass_exec_p.bind(
            *operands,
            out_avals=tuple(out_avals),
            in_names=tuple(all_in_names),
            out_names=tuple(out_names),
            lowering_input_output_aliases=(),
            sim_require_finite=True,
            sim_require_nnan=True,
            nc=nc,
        )
        return tuple(outs)

    devices = jax.devices()[:N_CORES]
    mesh = Mesh(np.asarray(devices), ("core",))
    n_in = len(in_names) + len(zero_outs)
    fn = jax.jit(
        shard_map(_body, mesh=mesh,
                  in_specs=(PartitionSpec("core"),) * n_in,
                  out_specs=(PartitionSpec("core"),) * len(out_names),
                  check_rep=False),
        keep_unused=True,
    )
    _CACHE["r"] = (fn, mesh, in_names, out_names, out_avals, zero_outs)
    return _CACHE["r"]


def _prep_in_maps(X, Wq1, Wk1, Wv1, Wo1, Wq2, Wk2, Wv2, Wo2, ln_g, ln_b,
                  W1, b1, W2, b2):
    h16 = lambda a: np.ascontiguousarray(np.asarray(a), dtype=np.float16)
    f32c = lambda a: np.ascontiguousarray(np.asarray(a), dtype=np.float32)
    shared = {
        "Wq1": h16(Wq1), "Wk1": h16(Wk1), "Wv1": h16(Wv1), "Wo1": h16(Wo1),
        "Wq2": h16(Wq2), "Wk2": h16(Wk2), "Wv2": h16(Wv2), "Wo2": h16(Wo2),
        "W1C": h16(np.asarray(W1).reshape(D, F // P, P).transpose(1, 0, 2)),
        "W2": h16(W2),
        "B1": f32c(np.asarray(b1).reshape(F // P, P).T),
        "B2": f32c(np.asarray(b2).reshape(KT, P).T),
        "G": f32c(np.asarray(ln_g).reshape(KT, P).T),
        "BB": f32c(np.asarray(ln_b).reshape(KT, P).T),
        "TRIU": np.triu(np.ones((P, P), np.float16)),
    }
    Xn = np.asarray(X)
    in_maps = []
    for c in range(N_CORES):
        m = dict(shared)
        m["XT"] = h16(Xn[c // 2].T)
        in_maps.append(m)
    return in_maps


def kernel(**inputs) -> np.ndarray:
    import jax

    fn, mesh, in_names, out_names, out_avals, zero_outs = _get_runner()
    in_maps = _prep_in_maps(**inputs)
    concat_in = [
        np.concatenate([in_maps[c][name] for c in range(N_CORES)], axis=0)
        for name in in_names
    ]
    concat_zeros = [
        np.zeros((N_CORES * z.shape[0], *z.shape[1:]), z.dtype)
        for z in zero_outs
    ]
    outs = fn(*concat_in, *concat_zeros)
    jax.block_until_ready(outs)
    i_out = out_names.index("OUT")
    per_core = np.asarray(outs[i_out]).reshape(N_CORES, D, SH)
    # assemble: core c -> batch c//2, query half c%2 ; output is [B, S, D]
    result = np.empty((B, S, D), np.float32)
    for c in range(N_CORES):
        b, h = c // 2, c % 2
        result[b, h * SH:(h + 1) * SH, :] = per_core[c].T
    return result
